# revision 1
# baseline (speedup 1.0000x reference)
"""Trainium2 Bass kernel for nn_ByteEncoder.

Model (see harness reference): byte + 6 n-gram hash embeddings summed -> one
post-norm transformer encoder layer (MHA + relu FFN) -> cross-attention from
patch-boundary queries to the full sequence.

Sharding: 8 cores; core c handles batch b=c//2, sequence half h=c%2
(1024 tokens).  The ~1.2GB embedding tables are replicated per core and
gathered on-device via indirect DMA (accumulating over the 7 tables with the
SDMA inline adder).  Self-attn K/V and the layer output x2 are exchanged
between the two cores of a batch with pair-wise AllGather collectives.
All matmuls run on fp32 data rounded to float32r (full-rate on the PE), except
the attention-probability matmuls which use bf16 (error washes out in the
2048-wide softmax averaging).
"""

import sys
import numpy as np

sys.path.insert(0, "/opt/trn_rl_repo")

import concourse.bass as bass
import concourse.bacc as bacc
import concourse.tile as tile
import concourse.mybir as mybir
from concourse.bass_utils import run_bass_kernel_spmd
from concourse.masks import make_identity
from concourse.tile import add_dep_helper

F32 = mybir.dt.float32
F32R = mybir.dt.float32r
BF16 = mybir.dt.bfloat16
I32 = mybir.dt.int32
AF = mybir.ActivationFunctionType

B, S, D, H, V, P = 4, 2048, 512, 8, 100000, 256
NGRAMS = list(range(3, 9))
NT = 1 + len(NGRAMS)          # 7 tables (byte + 6 ngram)
DH = D // H                   # 64
DF = 4 * D                    # 2048
SCALE = float(np.float32(DH) ** -0.5)
N_CORES = 8
SL = S // 2                   # 1024 local tokens
PL = P // 2                   # 128 local queries
KT = D // 128                 # 4 k-tiles over D
TT_L = SL // 128              # 8 local token tiles
TT_F = S // 128               # 16 full token tiles
FT = DF // 128                # 16 tiles over d_ff
VROWS = 256 + len(NGRAMS) * V # combined table rows

# DRAM f32-element offsets inside the kv / x2 bounce buffers
KT_ELE = D * SL                        # 524288 f32 (K^T block)
V1_ELE = 128 * TT_L * H * (DH + 1) // 2  # bf16 V' block as f32 elems = 266240
KV_ELE = KT_ELE + V1_ELE
X2T_ELE = D * SL                       # X2^T block
X2_ELE = SL * D                        # token-major x2 block
X2B_ELE = X2T_ELE + X2_ELE

_W512 = ["sWq", "sWk", "sWv", "sWo", "cWq", "cWk", "cWv", "cWo"]
_BVEC = ["sbq", "sbk", "sbv", "sbo", "b2", "cbq", "cbk", "cbv", "cbo",
         "ln1g", "ln1b", "ln2g", "ln2b"]


def _build_program(stage="H", vrows=VROWS):
    nc = bacc.Bacc("TRN2", target_bir_lowering=False, debug=False,
                   num_devices=N_CORES)
    dt = {}
    dt["table"] = nc.dram_tensor("table", [vrows, D], F32, kind="ExternalInput").ap()
    dt["idx"] = nc.dram_tensor("idx", [128, NT, TT_L], I32, kind="ExternalInput").ap()
    dt["qoff"] = nc.dram_tensor("qoff", [128, 1], I32, kind="ExternalInput").ap()
    for w in _W512:
        dt[w] = nc.dram_tensor(w, [D, D], F32, kind="ExternalInput").ap()
    dt["W1"] = nc.dram_tensor("W1", [D, DF], F32, kind="ExternalInput").ap()
    dt["W2"] = nc.dram_tensor("W2", [DF, D], F32, kind="ExternalInput").ap()
    dt["b1"] = nc.dram_tensor("b1", [DF], F32, kind="ExternalInput").ap()
    for bv in _BVEC:
        dt[bv] = nc.dram_tensor(bv, [D], F32, kind="ExternalInput").ap()
    out_d = nc.dram_tensor("out", [PL, D], F32, kind="ExternalOutput").ap()

    # DRAM bounce buffers for the pair collectives
    kv_in = nc.dram_tensor("kv_in", [KV_ELE], F32, kind="Internal").ap()
    kv_all = nc.dram_tensor("kv_all", [2, KV_ELE], F32, kind="Internal").ap()
    x2_in = nc.dram_tensor("x2_in", [X2B_ELE], F32, kind="Internal").ap()
    x2_all = nc.dram_tensor("x2_all", [2, X2B_ELE], F32, kind="Internal").ap()
    groups = [[0, 1], [2, 3], [4, 5], [6, 7]]

    with tile.TileContext(nc) as tc:
        _emit(nc, tc, dt, out_d, kv_in, kv_all, x2_in, x2_all, groups, stage)
    nc.compile()
    return nc


def _mm_acc(nc, ps, lhsT_tiles, rhs_tiles):
    n = len(lhsT_tiles)
    for k in range(n):
        nc.tensor.matmul(ps, lhsT=lhsT_tiles[k], rhs=rhs_tiles[k],
                         start=(k == 0), stop=(k == n - 1))


def _emit(nc, tc, dt, out_d, kv_in, kv_all, x2_in, x2_all, groups, stage="H"):
    from contextlib import ExitStack

    ctx = ExitStack()
    with ctx:
        # One big pool; tensors with disjoint lifetimes share a slot via the
        # same tag (bufs=1 -> strict sequential reuse, enforced by tile deps).
        big = ctx.enter_context(tc.tile_pool(name="big", bufs=1))
        pers = ctx.enter_context(tc.tile_pool(name="pers", bufs=1))
        pExp = ctx.enter_context(tc.tile_pool(name="pExp", bufs=3))
        psT = ctx.enter_context(tc.tile_pool(name="psT", bufs=2, space="PSUM"))
        ps512 = ctx.enter_context(tc.tile_pool(name="ps512", bufs=2, space="PSUM"))
        psAV = ctx.enter_context(tc.tile_pool(name="psAV", bufs=2, space="PSUM"))
        psC = ctx.enter_context(tc.tile_pool(name="psC", bufs=1, space="PSUM"))

        identF = pers.tile([128, 128], F32)
        make_identity(nc, identF[:])
        epsT = pers.tile([128, 1], F32)
        nc.vector.memset(epsT[:], 1e-5)
        ones64 = pers.tile([128, TT_F * H], F32)
        nc.vector.memset(ones64[:], 1.0)

        # broadcast-along-free bias rows, two chained 4-row slots
        def load_bcast(tile_, i, name):
            src = dt[name]
            bc_ap = bass.AP(tensor=src.tensor, offset=src.offset,
                            ap=[[0, 128]] + list(src.ap))
            nc.gpsimd.dma_start(out=tile_[:, i, :], in_=bc_ap)
            return tile_[:, i, :]

        bc1 = big.tile([128, 4, D], F32, tag="bc")
        bcast = {}
        for i, name in enumerate(["sbv", "sbo", "ln1g", "ln1b"]):
            bcast[name] = load_bcast(bc1, i, name)
        # per-partition (feature-major) bias tiles
        pp = {}
        for name in ["sbq", "sbk", "cbq", "cbk", "b2"]:
            t = pers.tile([128, KT], F32, tag=f"pp_{name}")
            nc.sync.dma_start(out=t[:], in_=dt[name].rearrange("(dp p) -> p dp", p=128))
            pp[name] = t
        b1_s = pers.tile([128, FT], F32)
        nc.sync.dma_start(out=b1_s[:], in_=dt["b1"].rearrange("(dp p) -> p dp", p=128))

        # self-attn QKV weights: one 24KB tile in the s32a chain slot
        sWqkv = big.tile([128, 3, KT, D], F32R, tag="s32a")
        for i, name in enumerate(["sWq", "sWk", "sWv"]):
            nc.sync.dma_start(
                out=sWqkv[:, i, :, :],
                in_=dt[name].bitcast(F32R).rearrange("(kt p) n -> p kt n", p=128))
        sWq_s, sWk_s, sWv_s = sWqkv[:, 0], sWqkv[:, 1], sWqkv[:, 2]

        # ---------------- Phase A: gather + embeds + X^T ----------------
        idx_t = pers.tile([128, NT, TT_L], I32)
        nc.sync.dma_start(idx_t[:], dt["idx"][:])
        emb = big.tile([128, TT_L, D], F32, tag="s16c")
        for tt in range(TT_L):
            # HW indirect DMA gathers one row per partition per call
            emb7 = big.tile([128, NT, D], F32, tag=("s16a" if tt % 2 else "s32c"))
            for j in range(NT):
                nc.gpsimd.indirect_dma_start(
                    out=emb7[:, j, :], out_offset=None, in_=dt["table"][:],
                    in_offset=bass.IndirectOffsetOnAxis(ap=idx_t[:, j, tt:tt + 1], axis=0))
            nc.vector.tensor_add(emb[:, tt, :], emb7[:, 0, :], emb7[:, 1, :])
            for j in range(2, NT):
                nc.vector.tensor_add(emb[:, tt, :], emb[:, tt, :], emb7[:, j, :])
            nc.scalar.mul(emb[:, tt, :], emb[:, tt, :], 1.0 / NT)

        if stage == "A":
            nc.sync.dma_start(out_d[:], emb[:, 0, :])
            return
        XT = big.tile([128, KT, SL], F32R, tag="s32c")
        for tt in range(TT_L):
            for dp in range(KT):
                pt = psT.tile([128, 128], F32, tag="pt")
                nc.tensor.transpose(pt[:], emb[:, tt, dp * 128:(dp + 1) * 128], identF[:])
                nc.vector.tensor_copy(XT[:, dp, tt * 128:(tt + 1) * 128], pt[:])

        # ---------------- Phase B: QKV projections (local tokens) ----------------
        QT = big.tile([128, KT, SL], F32R, tag="s16a")
        KTl = big.tile([128, KT, SL], F32R, tag="s16b")
        V1l = big.tile([128, TT_L, H, DH + 1], BF16, tag="s32b")
        nc.vector.tensor_copy(
            V1l[:, :, :, DH:DH + 1].rearrange("p a b c -> p (a b c)"),
            ones64[:, 0:TT_L * H])
        for dst, w_s, b_s in ((QT, sWq_s, pp["sbq"]), (KTl, sWk_s, pp["sbk"])):
            for dp in range(KT):
                for c2 in range(SL // 512):
                    ps = ps512.tile([128, 512], F32, tag="ps512")
                    _mm_acc(nc, ps[:],
                            [w_s[:, k, dp * 128:(dp + 1) * 128] for k in range(KT)],
                            [XT[:, k, c2 * 512:(c2 + 1) * 512] for k in range(KT)])
                    nc.scalar.activation(dst[:, dp, c2 * 512:(c2 + 1) * 512], ps[:],
                                         AF.Identity, bias=b_s[:, dp:dp + 1])
        for tt in range(TT_L):
            ps = ps512.tile([128, 512], F32, tag="ps512")
            _mm_acc(nc, ps[:],
                    [XT[:, k, tt * 128:(tt + 1) * 128] for k in range(KT)],
                    [sWv_s[:, k, :] for k in range(KT)])
            nc.vector.tensor_add(
                V1l[:, tt, :, 0:DH],
                ps[:].rearrange("p (h d) -> p h d", h=H),
                bcast["sbv"].rearrange("p (h d) -> p h d", h=H))

        if stage == "B":
            nc.sync.dma_start(out_d[:].rearrange("p (a b) -> p a b", a=KT),
                              QT[:, :, 0:128].bitcast(F32))
            return
        if stage == "V":
            nc.gpsimd.dma_start(out_d[:].rearrange("p (a b) -> p a b", a=H)[:, :, 0:DH],
                                V1l[:, 0, :, 0:DH])
            return
        # ---------------- Phase C: AllGather K^T and V' ----------------
        nc.sync.dma_start(
            out=kv_in[0:KT_ELE].rearrange("(dp p t) -> p dp t", p=128, t=SL),
            in_=KTl[:].bitcast(F32))
        nc.sync.dma_start(
            out=kv_in[KT_ELE:KV_ELE].bitcast(BF16).rearrange("(p x) -> p x", p=128),
            in_=V1l[:].rearrange("p a b c -> p (a b c)"))
        nc.gpsimd.collective_compute(
            "AllGather", mybir.AluOpType.bypass, replica_groups=groups,
            ins=[kv_in.opt()], outs=[kv_all.opt()])
        KTf = big.tile([128, KT, S], F32R, tag="s32a")
        V1f = big.tile([128, TT_F, H, DH + 1], BF16, tag="s32c")
        for r in range(2):
            nc.sync.dma_start(
                out=KTf[:, :, r * SL:(r + 1) * SL],
                in_=kv_all[r, 0:KT_ELE].bitcast(F32R).rearrange(
                    "(dp p t) -> p dp t", p=128, t=SL))
            nc.sync.dma_start(
                out=V1f[:, r * TT_L:(r + 1) * TT_L, :, :],
                in_=kv_all[r, KT_ELE:KV_ELE].bitcast(BF16).rearrange(
                    "(p a b c) -> p a b c", p=128, a=TT_L, b=H))

        if stage == "C":
            nc.sync.dma_start(out_d[:].rearrange("p (a b) -> p a b", a=KT),
                              KTf[:, :, SL:SL + 128].bitcast(F32))
            return
        if stage == "W":
            nc.gpsimd.dma_start(out_d[:].rearrange("p (a b) -> p a b", a=H)[:, :, 0:DH],
                                V1f[:, TT_L, :, 0:DH])
            return
        # ---------------- Phase D: self-attention ----------------
        if stage == "S":
            psS = ps512.tile([128, 512], F32, tag="ps512")
            nc.tensor.matmul(psS[:], lhsT=KTf[0:DH, 0, 0:128], rhs=QT[0:DH, 0, 0:512],
                             start=True, stop=True)
            eT = pExp.tile([128, SL], BF16, tag="expT")
            nc.scalar.activation(eT[:, 0:512], psS[:], AF.Exp, scale=SCALE)
            nc.gpsimd.dma_start(out_d[:], eT[:, 0:512])
            return
        O_tok = big.tile([128, TT_L, D], F32R, tag="s16d")
        for h in range(H):
            hp, hr = h // 2, (h % 2) * DH
            avA = psAV.tile([128, 4, DH + 1], F32, tag="av")
            avB = psAV.tile([128, 4, DH + 1], F32, tag="av")
            for tkt in range(TT_F):
                expT = pExp.tile([128, SL], BF16, tag="expT")
                for c2 in range(SL // 512):
                    psS = ps512.tile([128, 512], F32, tag="ps512")
                    nc.tensor.matmul(
                        psS[:],
                        lhsT=KTf[hr:hr + DH, hp, tkt * 128:(tkt + 1) * 128],
                        rhs=QT[hr:hr + DH, hp, c2 * 512:(c2 + 1) * 512],
                        start=True, stop=True)
                    nc.scalar.activation(expT[:, c2 * 512:(c2 + 1) * 512], psS[:],
                                         AF.Exp, scale=SCALE)
                for tqt in range(TT_L):
                    av = (avA if tqt < 4 else avB)[:, tqt % 4, :]
                    nc.tensor.matmul(
                        av, lhsT=expT[:, tqt * 128:(tqt + 1) * 128],
                        rhs=V1f[:, tkt, h, :],
                        start=(tkt == 0), stop=(tkt == TT_F - 1))
            if stage == "R" and h == 0:
                dmp = pers.tile([128, 260], F32, tag="dmp")
                nc.vector.tensor_copy(dmp[:].rearrange("p (a b) -> p a b", a=4), avA[:])
                nc.sync.dma_start(out_d[:, 0:260], dmp[:])
                return
            for tqt in range(TT_L):
                av = (avA if tqt < 4 else avB)[:, tqt % 4, :]
                rcp = pers.tile([128, 1], F32, tag="rcp")
                nc.vector.reciprocal(rcp[:], av[:, DH:DH + 1])
                nc.vector.tensor_scalar_mul(
                    O_tok[:, tqt, h * DH:(h + 1) * DH], in0=av[:, 0:DH], scalar1=rcp[:])

        if stage == "D":
            nc.sync.dma_start(out_d[:], O_tok[:, 0, :].bitcast(F32))
            return
        # ---------------- Phase E: O^T, O-proj, +emb, LN1 ----------------
        sWo_s = big.tile([128, KT, D], F32R, tag="s8")
        nc.sync.dma_start(
            out=sWo_s[:], in_=dt["sWo"].bitcast(F32R).rearrange("(kt p) n -> p kt n", p=128))
        OT = big.tile([128, KT, SL], F32R, tag="s16a")
        for tt in range(TT_L):
            for dp in range(KT):
                pt = psT.tile([128, 128], F32, tag="pt")
                nc.tensor.transpose(pt[:], O_tok[:, tt, dp * 128:(dp + 1) * 128].bitcast(F32), identF[:])
                nc.vector.tensor_copy(OT[:, dp, tt * 128:(tt + 1) * 128], pt[:].bitcast(F32R))
        x1 = big.tile([128, TT_L, D], F32, tag="s16b")
        for tt in range(TT_L):
            ps = ps512.tile([128, 512], F32, tag="ps512")
            _mm_acc(nc, ps[:],
                    [OT[:, k, tt * 128:(tt + 1) * 128] for k in range(KT)],
                    [sWo_s[:, k, :] for k in range(KT)])
            t0 = pers.tile([128, D], F32, tag="lnt0")
            nc.vector.tensor_add(t0[:], ps[:], bcast["sbo"])
            nc.vector.tensor_add(t0[:], t0[:], emb[:, tt, :])
            _layernorm(nc, pers, x1[:, tt, :], t0[:], bcast["ln1g"], bcast["ln1b"], epsT)
        X1T = big.tile([128, KT, SL], F32R, tag="s16c")
        for tt in range(TT_L):
            for dp in range(KT):
                pt = psT.tile([128, 128], F32, tag="pt")
                nc.tensor.transpose(pt[:], x1[:, tt, dp * 128:(dp + 1) * 128], identF[:])
                nc.vector.tensor_copy(X1T[:, dp, tt * 128:(tt + 1) * 128], pt[:])

        if stage == "E":
            nc.sync.dma_start(out_d[:], x1[:, 0, :])
            return
        # ---------------- Phase F: FFN + LN2 -> x2, X2T ----------------
        bc2 = big.tile([128, 4, D], F32, tag="bc")
        for i, name in enumerate(["ln2g", "ln2b", "cbv", "cbo"]):
            bcast[name] = load_bcast(bc2, i, name)
        W1_s = big.tile([128, KT, DF], F32R, tag="s32a")
        nc.sync.dma_start(
            out=W1_s[:], in_=dt["W1"].bitcast(F32R).rearrange("(kt p) n -> p kt n", p=128))
        W2_s = big.tile([128, FT, D], F32R, tag="s32b")
        nc.sync.dma_start(
            out=W2_s[:], in_=dt["W2"].bitcast(F32R).rearrange("(kt p) n -> p kt n", p=128))
        x2 = big.tile([128, TT_L, D], F32, tag="s16d")
        X2T = big.tile([128, KT, SL], F32R, tag="s16a")
        for c2 in range(SL // 512):
            HT = big.tile([128, FT, 512], F32R, tag="s32c")
            for ft in range(FT):
                ps = ps512.tile([128, 512], F32, tag="ps512")
                _mm_acc(nc, ps[:],
                        [W1_s[:, k, ft * 128:(ft + 1) * 128] for k in range(KT)],
                        [X1T[:, k, c2 * 512:(c2 + 1) * 512] for k in range(KT)])
                nc.scalar.activation(HT[:, ft, :], ps[:], AF.Relu,
                                     bias=b1_s[:, ft:ft + 1])
            for dp in range(KT):
                ps = ps512.tile([128, 512], F32, tag="ps512")
                _mm_acc(nc, ps[:],
                        [W2_s[:, k, dp * 128:(dp + 1) * 128] for k in range(FT)],
                        [HT[:, k, :] for k in range(FT)])
                fft = pers.tile([128, 512], F32, tag="fft")
                nc.scalar.activation(fft[:], ps[:], AF.Identity, bias=pp["b2"][:, dp:dp + 1])
                for st in range(4):
                    tt = c2 * 4 + st
                    pt = psT.tile([128, 128], F32, tag="pt")
                    nc.tensor.transpose(pt[:], fft[:, st * 128:(st + 1) * 128], identF[:])
                    nc.vector.tensor_add(x2[:, tt, dp * 128:(dp + 1) * 128], pt[:],
                                         x1[:, tt, dp * 128:(dp + 1) * 128])
        for tt in range(TT_L):
            _layernorm(nc, pers, x2[:, tt, :], x2[:, tt, :], bcast["ln2g"],
                       bcast["ln2b"], epsT)
            for dp in range(KT):
                pt = psT.tile([128, 128], F32, tag="pt")
                nc.tensor.transpose(pt[:], x2[:, tt, dp * 128:(dp + 1) * 128], identF[:])
                nc.vector.tensor_copy(X2T[:, dp, tt * 128:(tt + 1) * 128], pt[:])

        if stage == "F":
            nc.sync.dma_start(out_d[:], x2[:, 0, :])
            return
        # ---------------- Phase G: AllGather x2 ----------------
        nc.sync.dma_start(
            out=x2_in[0:X2T_ELE].rearrange("(dp p t) -> p dp t", p=128, t=SL),
            in_=X2T[:].bitcast(F32))
        nc.sync.dma_start(
            out=x2_in[X2T_ELE:X2B_ELE].rearrange("(tt p d) -> p tt d", p=128, d=D),
            in_=x2[:])
        nc.gpsimd.collective_compute(
            "AllGather", mybir.AluOpType.bypass, replica_groups=groups,
            ins=[x2_in.opt()], outs=[x2_all.opt()])
        X2Tf = big.tile([128, KT, S], F32R, tag="s32a")
        for r in range(2):
            nc.sync.dma_start(
                out=X2Tf[:, :, r * SL:(r + 1) * SL],
                in_=x2_all[r, 0:X2T_ELE].bitcast(F32R).rearrange(
                    "(dp p t) -> p dp t", p=128, t=SL))
        # gather the 128 local patch queries from the full token-major x2
        qoff_t = pers.tile([128, 1], I32)
        nc.sync.dma_start(qoff_t[:], dt["qoff"][:])
        qg = pers.tile([128, D], F32, tag="qg")
        nc.gpsimd.indirect_dma_start(
            out=qg[:], out_offset=None,
            in_=x2_all[:].rearrange("r e -> (r e)").rearrange("(n d) -> n d", d=D),
            in_offset=bass.IndirectOffsetOnAxis(ap=qoff_t[:, 0:1], axis=0))
        qT = pers.tile([128, KT, 128], F32R, tag="qT")
        for dp in range(KT):
            pt = psT.tile([128, 128], F32, tag="pt")
            nc.tensor.transpose(pt[:], qg[:, dp * 128:(dp + 1) * 128], identF[:])
            nc.vector.tensor_copy(qT[:, dp, :], pt[:])

        if stage == "G":
            nc.sync.dma_start(out_d[:], qg[:])
            return
        # ---------------- Phase H: cross-attention ----------------
        cWall = big.tile([128, 4, KT, D], F32R, tag="s32c")
        for i, name in enumerate(["cWq", "cWk", "cWv", "cWo"]):
            nc.sync.dma_start(
                out=cWall[:, i, :, :],
                in_=dt[name].bitcast(F32R).rearrange("(kt p) n -> p kt n", p=128))
        cWq_s, cWk_s, cWv_s, cWo_s = (cWall[:, i] for i in range(4))
        cQT = pers.tile([128, KT, 128], BF16, tag="cQT")
        cQsb = pers.tile([128, D], F32, tag="cQsb")
        ps = ps512.tile([128, 512], F32, tag="ps512")
        _mm_acc(nc, ps[:],
                [qT[:, k, :] for k in range(KT)],
                [cWq_s[:, k, :] for k in range(KT)])
        nc.vector.tensor_copy(cQsb[:], ps[:])
        for dp in range(KT):
            pt = psT.tile([128, 128], F32, tag="pt")
            nc.tensor.transpose(pt[:], cQsb[:, dp * 128:(dp + 1) * 128], identF[:])
            nc.scalar.activation(cQT[:, dp, :], pt[:], AF.Identity,
                                 bias=pp["cbq"][:, dp:dp + 1])
        cKTf = big.tile([128, KT, S], BF16, tag="s16a")
        for dp in range(KT):
            for c4 in range(S // 512):
                ps = ps512.tile([128, 512], F32, tag="ps512")
                _mm_acc(nc, ps[:],
                        [cWk_s[:, k, dp * 128:(dp + 1) * 128] for k in range(KT)],
                        [X2Tf[:, k, c4 * 512:(c4 + 1) * 512] for k in range(KT)])
                nc.scalar.activation(cKTf[:, dp, c4 * 512:(c4 + 1) * 512], ps[:],
                                     AF.Identity, bias=pp["cbk"][:, dp:dp + 1])
        cV1f = big.tile([128, TT_F, H, DH + 1], F32, tag="s32b")
        nc.vector.tensor_copy(
            cV1f[:, :, :, DH:DH + 1].rearrange("p a b c -> p (a b c)"),
            ones64[:])
        for tt in range(TT_F):
            ps = ps512.tile([128, 512], F32, tag="ps512")
            _mm_acc(nc, ps[:],
                    [X2Tf[:, k, tt * 128:(tt + 1) * 128] for k in range(KT)],
                    [cWv_s[:, k, :] for k in range(KT)])
            nc.vector.tensor_add(
                cV1f[:, tt, :, 0:DH],
                ps[:].rearrange("p (h d) -> p h d", h=H),
                bcast["cbv"].rearrange("p (h d) -> p h d", h=H))
        Oc = pers.tile([128, D], F32R, tag="Oc")
        for h in range(H):
            hp, hr = h // 2, (h % 2) * DH
            avc = psC.tile([128, 1, DH + 1], F32, tag="avc")
            for tkt in range(TT_F):
                psc = psC.tile([128, 128], F32, tag="psc")
                nc.tensor.matmul(
                    psc[:], lhsT=cKTf[hr:hr + DH, hp, tkt * 128:(tkt + 1) * 128],
                    rhs=cQT[hr:hr + DH, hp, :], start=True, stop=True)
                ec = pers.tile([128, 128], F32, tag="ec")
                nc.scalar.activation(ec[:], psc[:], AF.Exp, scale=SCALE)
                nc.tensor.matmul(
                    avc[:, 0, :], lhsT=ec[:], rhs=cV1f[:, tkt, h, :],
                    start=(tkt == 0), stop=(tkt == TT_F - 1))
            rcp = pers.tile([128, 1], F32, tag="rcp")
            nc.vector.reciprocal(rcp[:], avc[:, 0, DH:DH + 1])
            nc.vector.tensor_scalar_mul(Oc[:, h * DH:(h + 1) * DH],
                                        in0=avc[:, 0, 0:DH], scalar1=rcp[:])
        OcT = pers.tile([128, KT, 128], F32R, tag="OcT")
        for dp in range(KT):
            pt = psT.tile([128, 128], F32, tag="pt")
            nc.tensor.transpose(pt[:], Oc[:, dp * 128:(dp + 1) * 128].bitcast(F32), identF[:])
            nc.vector.tensor_copy(OcT[:, dp, :], pt[:].bitcast(F32R))
        ps = ps512.tile([128, 512], F32, tag="ps512")
        _mm_acc(nc, ps[:],
                [OcT[:, k, :] for k in range(KT)],
                [cWo_s[:, k, :] for k in range(KT)])
        outsb = pers.tile([128, D], F32, tag="outsb")
        nc.vector.tensor_add(outsb[:], ps[:], bcast["cbo"])
        nc.sync.dma_start(out_d[:], outsb[:])


def _layernorm(nc, pool, out_ap, in_ap, g_b, b_b, epsT):
    st = pool.tile([128, 6], F32, tag="ln_st")
    nc.vector.bn_stats(out=st[:], in_=in_ap)
    mv = pool.tile([128, 2], F32, tag="ln_mv")
    nc.vector.bn_aggr(out=mv[:], in_=st[:])
    sd = pool.tile([128, 1], F32, tag="ln_sd")
    nc.scalar.activation(sd[:], mv[:, 1:2], AF.Sqrt, bias=epsT[:])
    nc.vector.reciprocal(sd[:], sd[:])
    tmp = pool.tile([128, D], F32, tag="ln_tmp")
    nc.vector.tensor_scalar(out=tmp[:], in0=in_ap, scalar1=mv[:, 0:1], scalar2=sd[:],
                            op0=mybir.AluOpType.subtract, op1=mybir.AluOpType.mult)
    nc.vector.tensor_mul(tmp[:], tmp[:], g_b[:])
    nc.vector.tensor_add(out_ap, tmp[:], b_b[:])


def _ngram_hashes(bytes_seq):
    """int64-wraparound n-gram hashes, mod V.  [B, S] -> [len(NGRAMS), B, S]"""
    b = bytes_seq.astype(np.int64)
    out = np.zeros((len(NGRAMS), b.shape[0], S), dtype=np.int64)
    for j, n in enumerate(NGRAMS):
        h = np.zeros_like(b)
        for k in range(n):
            shift = n - 1 - k
            mult = np.int64(256) ** k  # wraps for n=8, matching torch/jax int64
            shifted = np.zeros_like(b)
            shifted[:, shift:] = b[:, : S - shift]
            h = h + shifted * mult
        h = np.where(np.arange(S)[None, :] >= (n - 1), h, 0)
        out[j] = h % V
    return out


_PROGRAM = None


def _get_program():
    global _PROGRAM
    if _PROGRAM is None:
        _PROGRAM = _build_program()
    return _PROGRAM


def make_in_maps(inputs):
    bytes_seq = np.asarray(inputs["bytes_seq"])
    patch_idx = np.asarray(inputs["patch_idx"])
    byte_emb = np.asarray(inputs["byte_emb"], dtype=np.float32)
    ngram_emb = np.asarray(inputs["ngram_emb"], dtype=np.float32)

    table = np.concatenate([byte_emb, ngram_emb.reshape(len(NGRAMS) * V, D)], axis=0)
    assert table.shape == (VROWS, D)
    hashes = _ngram_hashes(bytes_seq)

    weights = {}
    for w in _W512 + ["W1", "W2", "b1"] + _BVEC:
        key = {"b2": "b2"}.get(w, w)
        weights[w] = np.ascontiguousarray(np.asarray(inputs[key], dtype=np.float32))

    in_maps = []
    for c in range(N_CORES):
        b, hh = c // 2, c % 2
        tok0 = hh * SL
        # idx[p, j, tt] = combined-table row for token tok0 + tt*128 + p, table j
        t = tok0 + np.arange(TT_L)[None, :] * 128 + np.arange(128)[:, None, None] * 0
        # build explicitly:
        p_ar = np.arange(128)[:, None]          # [128, 1]
        tt_ar = np.arange(TT_L)[None, :]        # [1, TT_L]
        tok = tok0 + tt_ar * 128 + p_ar         # [128, TT_L]
        idx = np.zeros((128, NT, TT_L), dtype=np.int32)
        idx[:, 0, :] = bytes_seq[b][tok].astype(np.int32)
        for j in range(len(NGRAMS)):
            idx[:, 1 + j, :] = (256 + j * V + hashes[j, b][tok]).astype(np.int32)
        # query rows into the flat x2_all viewed [4096, D]:
        # global token g -> (g//SL)*2*SL + SL + (g%SL)   (X2T block precedes rows)
        g = patch_idx[b, hh * PL: (hh + 1) * PL].astype(np.int64)
        qoff = ((g // SL) * (2 * SL) + SL + (g % SL)).astype(np.int32)[:, None]
        m = {"table": table, "idx": idx, "qoff": qoff}
        m.update(weights)
        in_maps.append(m)
    return in_maps


def assemble_output(results):
    out = np.zeros((B, P, D), dtype=np.float32)
    for c in range(N_CORES):
        b, hh = c // 2, c % 2
        out[b, hh * PL:(hh + 1) * PL, :] = results[c]["out"]
    return out


def kernel(**inputs):
    nc = _get_program()
    in_maps = make_in_maps(inputs)
    res = run_bass_kernel_spmd(nc, in_maps, core_ids=list(range(N_CORES)))
    return assemble_output(res.results)


if __name__ == "__main__":
    pass



# revision 2
# speedup vs baseline: 3.2187x; 3.2187x over previous
"""Trainium2 Bass kernel for nn_ByteEncoder (optimized v2).

Model: byte + 6 n-gram hash embeddings summed -> one post-norm transformer
encoder layer (MHA + relu FFN) -> cross-attention from patch-boundary queries
to the full sequence.

Sharding: 8 cores; core c handles batch b=c//2, sequence half h=c%2 (1024
tokens).  All matmul operands bf16 (f32 PSUM accumulate).  The embedding
tables are gathered with the SDMA inline adder (7 accumulating indirect
DMAs).  Self-attention runs local keys first so the pairwise K/V AllGather
overlaps with compute; the attention A@V is computed feature-major
(out = V'^T E) with a ones-column producing the softmax denominators.
Cross-attention is key-split across the pair: only the 256 patch-query rows
(256KB) and the partial attention accumulators (532KB) are exchanged instead
of the full x2 activations.
"""

import sys
import numpy as np

sys.path.insert(0, "/opt/trn_rl_repo")

import concourse.bass as bass
import concourse.bacc as bacc
import concourse.tile as tile
import concourse.mybir as mybir
from concourse.bass_utils import run_bass_kernel_spmd
from concourse.masks import make_identity

F32 = mybir.dt.float32
F32R = mybir.dt.float32r
BF16 = mybir.dt.bfloat16
I32 = mybir.dt.int32
AF = mybir.ActivationFunctionType
ALU = mybir.AluOpType

B, S, D, H, V, P = 4, 2048, 512, 8, 100000, 256
NGRAMS = list(range(3, 9))
NT = 1 + len(NGRAMS)          # 7 tables (byte + 6 ngram)
DH = D // H                   # 64
DF = 4 * D                    # 2048
SCALE = float(np.float32(DH) ** -0.5)
N_CORES = 8
SL = S // 2                   # 1024 local tokens
PL = P // 2                   # 128 local queries
KT = D // 128                 # 4 feature blocks
TT_L = SL // 128              # 8 local token tiles
FT = DF // 128                # 16 tiles over d_ff
VROWS = 256 + len(NGRAMS) * V

# DRAM exchange blob geometry (all byte-counted as f32 elements for the
# collective tensors; bf16 payload is bitcast)
K_ROW = KT * SL               # 4096 bf16 per partition-row
V_ROW = TT_L * H * (DH + 1)   # 4160 bf16 per partition-row
Q_ROW = D                     # 512 bf16 per gathered query row
PR_ROWS = H * (DH + 1)        # 520 f32 rows in the partial blob
PR_COL = 2 * PL               # 256 queries per batch

_W512 = ["sWq", "sWk", "sWv", "sWo", "cWq", "cWk", "cWv", "cWo"]


def _build_program(stage="Z"):
    nc = bacc.Bacc("TRN2", target_bir_lowering=False, debug=False,
                   num_devices=N_CORES)
    dt = {}
    dt["table"] = nc.dram_tensor("table", [VROWS, D], BF16, kind="ExternalInput").ap()
    dt["idx"] = nc.dram_tensor("idx", [128, NT, TT_L], I32, kind="ExternalInput").ap()
    dt["kr_idx"] = nc.dram_tensor("kr_idx", [128, 1], I32, kind="ExternalInput").ap()
    dt["qc_idx"] = nc.dram_tensor("qc_idx", [128, 2], I32, kind="ExternalInput").ap()
    dt["qa_idx"] = nc.dram_tensor("qa_idx", [128, 2], I32, kind="ExternalInput").ap()
    dt["pr_idx"] = nc.dram_tensor("pr_idx", [128, H], I32, kind="ExternalInput").ap()
    for w in _W512:
        dt[w] = nc.dram_tensor(w, [D, D], BF16, kind="ExternalInput").ap()
    dt["W1"] = nc.dram_tensor("W1", [D, DF], BF16, kind="ExternalInput").ap()
    dt["W2"] = nc.dram_tensor("W2", [DF, D], BF16, kind="ExternalInput").ap()
    # per-partition (feature-major) f32 biases
    for bv in ["sbq", "sbk", "cbq", "cbk", "b2"]:
        dt[bv] = nc.dram_tensor(bv, [D], F32, kind="ExternalInput").ap()
    dt["b1"] = nc.dram_tensor("b1", [DF], F32, kind="ExternalInput").ap()
    # broadcast-row bf16 biases
    for bv in ["sbv", "sbo", "ln1g", "ln1b", "ln2g", "ln2b", "cbv", "cbo"]:
        dt[bv] = nc.dram_tensor(bv, [D], BF16, kind="ExternalInput").ap()
    out_d = nc.dram_tensor("out", [PL, D], F32, kind="ExternalOutput").ap()

    # DRAM bounce buffers for the pair collectives (f32-typed, bf16 payload)
    kv_in_k = nc.dram_tensor("kv_in_k", [128 * K_ROW // 2], F32, kind="Internal").ap()
    kv_all_k = nc.dram_tensor("kv_all_k", [2, 128 * K_ROW // 2], F32, kind="Internal").ap()
    kv_in_v = nc.dram_tensor("kv_in_v", [128 * V_ROW // 2], F32, kind="Internal").ap()
    kv_all_v = nc.dram_tensor("kv_all_v", [2, 128 * V_ROW // 2], F32, kind="Internal").ap()
    x2loc = nc.dram_tensor("x2loc", [SL * D // 2], F32, kind="Internal").ap()
    qx_in = nc.dram_tensor("qx_in", [PR_COL * Q_ROW // 2], F32, kind="Internal").ap()
    qx_all = nc.dram_tensor("qx_all", [2, PR_COL * Q_ROW // 2], F32, kind="Internal").ap()
    pr_in = nc.dram_tensor("pr_in", [PR_ROWS * PR_COL], F32, kind="Internal").ap()
    pr_all = nc.dram_tensor("pr_all", [2, PR_ROWS * PR_COL], F32, kind="Internal").ap()
    groups = [[0, 1], [2, 3], [4, 5], [6, 7]]

    with tile.TileContext(nc) as tc:
        _emit(nc, tc, dt, out_d, kv_in_k, kv_all_k, kv_in_v, kv_all_v,
              x2loc, qx_in, qx_all, pr_in, pr_all, groups, stage)
    nc.compile()
    return nc


def _mm_acc(nc, ps, lhsT_tiles, rhs_tiles):
    n = len(lhsT_tiles)
    for k in range(n):
        nc.tensor.matmul(ps, lhsT=lhsT_tiles[k], rhs=rhs_tiles[k],
                         start=(k == 0), stop=(k == n - 1))


def _emit(nc, tc, dt, out_d, kv_in_k, kv_all_k, kv_in_v, kv_all_v,
          x2loc, qx_in, qx_all, pr_in, pr_all, groups, stage="Z"):
    from contextlib import ExitStack

    ctx = ExitStack()
    with ctx:
        big = ctx.enter_context(tc.tile_pool(name="big", bufs=1))
        pers = ctx.enter_context(tc.tile_pool(name="pers", bufs=1))
        pExp = ctx.enter_context(tc.tile_pool(name="pExp", bufs=3))
        pTmp = ctx.enter_context(tc.tile_pool(name="pTmp", bufs=2))
        psMM = ctx.enter_context(tc.tile_pool(name="psMM", bufs=2, space="PSUM"))
        psAV = ctx.enter_context(tc.tile_pool(name="psAV", bufs=1, space="PSUM"))
        psT = ctx.enter_context(tc.tile_pool(name="psT", bufs=2, space="PSUM"))

        identB = pers.tile([128, 128], BF16)
        make_identity(nc, identB[:])
        epsT = pers.tile([128, 1], F32)
        nc.vector.memset(epsT[:], 1e-5)
        onesb = pers.tile([128, DH], BF16)
        nc.vector.memset(onesb[:], 1.0)
        onesf = pers.tile([128, DH], F32)
        nc.vector.memset(onesf[:], 1.0)

        # broadcast-along-partition bias rows (bf16)
        def load_bcast(tile_, i, name):
            src = dt[name]
            bc_ap = bass.AP(tensor=src.tensor, offset=src.offset,
                            ap=[[0, 128]] + list(src.ap))
            nc.gpsimd.dma_start(out=tile_[:, i, :], in_=bc_ap)
            return tile_[:, i, :]

        bc1 = pers.tile([128, 8, D], BF16, tag="bc")
        bcast = {}
        for i, name in enumerate(["sbv", "sbo", "ln1g", "ln1b",
                                  "ln2g", "ln2b", "cbv", "cbo"]):
            bcast[name] = load_bcast(bc1, i, name)
        # per-partition (feature-major) f32 bias tiles
        pp = {}
        for name in ["sbq", "sbk", "cbq", "cbk", "b2"]:
            t = pers.tile([128, KT], F32, tag=f"pp_{name}")
            nc.sync.dma_start(out=t[:], in_=dt[name].rearrange("(dp p) -> p dp", p=128))
            pp[name] = t
        b1_s = pers.tile([128, FT], F32)
        nc.sync.dma_start(out=b1_s[:], in_=dt["b1"].rearrange("(dp p) -> p dp", p=128))

        # self-attn QKV weights, feature-major bf16
        sWqkv = big.tile([128, 3, KT, D], BF16, tag="sWqkv")
        for i, name in enumerate(["sWq", "sWk", "sWv"]):
            nc.sync.dma_start(
                out=sWqkv[:, i, :, :],
                in_=dt[name].rearrange("(kt p) n -> p kt n", p=128))
        sWq_s, sWk_s, sWv_s = sWqkv[:, 0], sWqkv[:, 1], sWqkv[:, 2]

        # ---------------- Phase A: accumulate-gather + emb ----------------
        idx_t = pers.tile([128, NT, TT_L], I32)
        nc.sync.dma_start(idx_t[:], dt["idx"][:])
        emb = big.tile([128, TT_L, D], BF16, tag="emb")
        for tt in range(TT_L):
            for j in range(NT):
                nc.gpsimd.indirect_dma_start(
                    out=emb[:, tt, :], out_offset=None, in_=dt["table"][:],
                    in_offset=bass.IndirectOffsetOnAxis(ap=idx_t[:, j, tt:tt + 1], axis=0),
                    compute_op=(ALU.bypass if j == 0 else ALU.add))
            nc.vector.tensor_scalar_mul(emb[:, tt, :], in0=emb[:, tt, :],
                                        scalar1=1.0 / NT)

        if stage == "A":
            pad = pers.tile([128, D], F32, tag="dbg")
            nc.vector.tensor_copy(pad[:], emb[:, 0, :])
            nc.sync.dma_start(out_d[:], pad[:])
            return
        # X^T feature-major bf16
        XT = big.tile([128, KT, SL], BF16, tag="XT")
        for tt in range(TT_L):
            for dp in range(KT):
                pt = psT.tile([128, 128], BF16, tag="pt")
                nc.tensor.transpose(pt[:], emb[:, tt, dp * 128:(dp + 1) * 128], identB[:])
                nc.vector.tensor_copy(XT[:, dp, tt * 128:(tt + 1) * 128], pt[:])

        # ---------------- Phase B: K,V then exchange, then Q ----------------
        KTl = big.tile([128, KT, SL], BF16, tag="KTl")
        for dp in range(KT):
            for c2 in range(SL // 512):
                ps = psMM.tile([128, 512], F32, tag="mm")
                _mm_acc(nc, ps[:],
                        [sWk_s[:, k, dp * 128:(dp + 1) * 128] for k in range(KT)],
                        [XT[:, k, c2 * 512:(c2 + 1) * 512] for k in range(KT)])
                nc.vector.tensor_scalar_add(KTl[:, dp, c2 * 512:(c2 + 1) * 512],
                                            in0=ps[:], scalar1=pp["sbk"][:, dp:dp + 1])
        V1l = big.tile([128, TT_L, H, DH + 1], BF16, tag="V1l")
        nc.vector.memset(
            V1l[:, :, :, DH:DH + 1].rearrange("p a b c -> p (a b c)"), 1.0)
        for tt in range(TT_L):
            ps = psMM.tile([128, 512], F32, tag="mm")
            _mm_acc(nc, ps[:],
                    [XT[:, k, tt * 128:(tt + 1) * 128] for k in range(KT)],
                    [sWv_s[:, k, :] for k in range(KT)])
            nc.vector.tensor_add(
                V1l[:, tt, :, 0:DH],
                ps[:].rearrange("p (h d) -> p h d", h=H),
                bcast["sbv"].rearrange("p (h d) -> p h d", h=H))

        # ship local K^T and V' to the partner
        nc.sync.dma_start(
            out=kv_in_k.bitcast(BF16).rearrange("(p x) -> p x", p=128),
            in_=KTl[:].rearrange("p a b -> p (a b)"))
        nc.sync.dma_start(
            out=kv_in_v.bitcast(BF16).rearrange("(p x) -> p x", p=128),
            in_=V1l[:].rearrange("p a b c -> p (a b c)"))
        nc.gpsimd.collective_compute(
            "AllGather", ALU.bypass, replica_groups=groups,
            ins=[kv_in_k.opt()], outs=[kv_all_k.opt()])
        nc.gpsimd.collective_compute(
            "AllGather", ALU.bypass, replica_groups=groups,
            ins=[kv_in_v.opt()], outs=[kv_all_v.opt()])

        QT = big.tile([128, KT, SL], BF16, tag="QT")
        for dp in range(KT):
            for c2 in range(SL // 512):
                ps = psMM.tile([128, 512], F32, tag="mm")
                _mm_acc(nc, ps[:],
                        [sWq_s[:, k, dp * 128:(dp + 1) * 128] for k in range(KT)],
                        [XT[:, k, c2 * 512:(c2 + 1) * 512] for k in range(KT)])
                nc.vector.tensor_scalar_add(QT[:, dp, c2 * 512:(c2 + 1) * 512],
                                            in0=ps[:], scalar1=pp["sbq"][:, dp:dp + 1])

        if stage == "B":
            pad = pers.tile([128, D], F32, tag="dbg")
            nc.vector.tensor_copy(pad[:].rearrange("p (a b) -> p a b", a=KT),
                                  QT[:, :, 0:128])
            nc.sync.dma_start(out_d[:], pad[:])
            return

        # prefetch heavy later-phase weights while attention runs
        sWo_s = big.tile([128, KT, D], BF16, tag="sWo")
        nc.sync.dma_start(out=sWo_s[:],
                          in_=dt["sWo"].rearrange("(kt p) n -> p kt n", p=128))
        W1_s = big.tile([128, KT, DF], BF16, tag="W1")
        nc.sync.dma_start(out=W1_s[:],
                          in_=dt["W1"].rearrange("(kt p) n -> p kt n", p=128))
        W2_s = big.tile([128, FT, D], BF16, tag="W2")
        nc.sync.dma_start(out=W2_s[:],
                          in_=dt["W2"].rearrange("(kt p) n -> p kt n", p=128))
        # ---------------- Phase C: self-attention, local keys ----------------
        # OTP: unnormalized sum(exp*V)^T partials, f32. den8: denominators.
        OTP = big.tile([128, KT, SL], BF16, tag="OTP")
        denSt = big.tile([65, SL], F32, tag="denSt")
        den8l = big.tile([8, SL], F32, tag="den8l")
        den8r = big.tile([8, SL], F32, tag="den8r")

        def attn_half(KTx, V1x, local):
            for h in range(H):
                hp, hr = h // 2, (h % 2) * DH
                avt = psAV.tile([DH + 1, SL], F32, tag="avt")
                for tkt in range(TT_L):
                    psS = psMM.tile([128, SL], F32, tag="mm")
                    for j in range(SL // 512):
                        nc.tensor.matmul(
                            psS[:, j * 512:(j + 1) * 512],
                            lhsT=KTx[hr:hr + DH, hp, tkt * 128:(tkt + 1) * 128],
                            rhs=QT[hr:hr + DH, hp, j * 512:(j + 1) * 512],
                            start=True, stop=True)
                    eT = pExp.tile([128, SL], BF16, tag="expT")
                    nc.scalar.activation(eT[:], psS[:], AF.Exp, scale=SCALE)
                    for j in range(SL // 512):
                        nc.tensor.matmul(
                            avt[:, j * 512:(j + 1) * 512],
                            lhsT=V1x[:, tkt, h, :],
                            rhs=eT[:, j * 512:(j + 1) * 512],
                            start=(tkt == 0), stop=(tkt == TT_L - 1))
                if local:
                    nc.vector.tensor_copy(OTP[hr:hr + DH, hp, :], avt[0:DH, :])
                    nc.vector.tensor_copy(denSt[DH:DH + 1, :], avt[DH:DH + 1, :])
                    nc.sync.dma_start(den8l[h:h + 1, :], denSt[DH:DH + 1, :])
                else:
                    nc.vector.tensor_add(OTP[hr:hr + DH, hp, :],
                                         avt[0:DH, :], OTP[hr:hr + DH, hp, :])
                    nc.vector.tensor_copy(denSt[DH:DH + 1, :], avt[DH:DH + 1, :])
                    nc.sync.dma_start(den8r[h:h + 1, :], denSt[DH:DH + 1, :])

        attn_half(KTl, V1l, True)

        # ---------------- Phase D: remote keys ----------------
        kr_t = pers.tile([128, 1], I32)
        nc.sync.dma_start(kr_t[:], dt["kr_idx"][:])
        KTr = big.tile([128, KT, SL], BF16, tag="KTr")
        nc.gpsimd.indirect_dma_start(
            out=KTr[:].rearrange("p a b -> p (a b)"), out_offset=None,
            in_=kv_all_k.bitcast(BF16).rearrange("a (b x) -> (a b) x", x=K_ROW),
            in_offset=bass.IndirectOffsetOnAxis(ap=kr_t[:, 0:1], axis=0))
        V1r = big.tile([128, TT_L, H, DH + 1], BF16, tag="V1r")
        nc.gpsimd.indirect_dma_start(
            out=V1r[:].rearrange("p a b c -> p (a b c)"), out_offset=None,
            in_=kv_all_v.bitcast(BF16).rearrange("a (b x) -> (a b) x", x=V_ROW),
            in_offset=bass.IndirectOffsetOnAxis(ap=kr_t[:, 0:1], axis=0))

        attn_half(KTr, V1r, False)

        # normalization: OTn = OTP * (1/den) broadcast along features
        den8 = big.tile([8, SL], F32, tag="denSt", bufs=1)
        nc.vector.tensor_add(den8[:], den8l[:], den8r[:])
        rcp8 = big.tile([8, SL], F32, tag="den8r", bufs=1)
        nc.vector.reciprocal(rcp8[:], den8[:])
        OTn = big.tile([128, KT, SL], BF16, tag="OTn")
        for h in range(H):
            hp, hr = h // 2, (h % 2) * DH
            rcpRow = pTmp.tile([1, SL], F32, tag="lnt0")
            nc.sync.dma_start(rcpRow[:], rcp8[h:h + 1, :])
            for j in range(SL // 512):
                psB = psT.tile([DH, 512], F32, tag="pt")
                nc.tensor.matmul(
                    psB[:], lhsT=onesf.bitcast(F32R)[0:1, :],
                    rhs=rcpRow.bitcast(F32R)[0:1, j * 512:(j + 1) * 512],
                    start=True, stop=True)
                nc.vector.tensor_mul(OTn[hr:hr + DH, hp, j * 512:(j + 1) * 512],
                                     OTP[hr:hr + DH, hp, j * 512:(j + 1) * 512],
                                     psB[:])

        if stage == "C":
            pad = pers.tile([128, D], F32, tag="dbg")
            nc.vector.tensor_copy(pad[:].rearrange("p (a b) -> p a b", a=KT),
                                  OTn[:, :, 0:128])
            nc.sync.dma_start(out_d[:], pad[:])
            return

        # cross-attn weights reuse the (now dead) OTP slot
        cWall = big.tile([128, 4, KT, D], BF16, tag="OTP")
        for i, name in enumerate(["cWq", "cWk", "cWv", "cWo"]):
            nc.sync.dma_start(
                out=cWall[:, i, :, :],
                in_=dt[name].rearrange("(kt p) n -> p kt n", p=128))
        cWq_s, cWk_s, cWv_s, cWo_s = (cWall[:, i] for i in range(4))

        # ---------------- Phase E: O-proj, +emb, LN1 ----------------
        x1 = big.tile([128, TT_L, D], BF16, tag="x1")
        for tt in range(TT_L):
            ps = psMM.tile([128, 512], F32, tag="mm")
            _mm_acc(nc, ps[:],
                    [OTn[:, k, tt * 128:(tt + 1) * 128] for k in range(KT)],
                    [sWo_s[:, k, :] for k in range(KT)])
            t0 = pTmp.tile([128, D], F32, tag="lnt0")
            nc.vector.tensor_add(t0[:], ps[:], bcast["sbo"])
            nc.vector.tensor_add(t0[:], t0[:], emb[:, tt, :])
            _layernorm(nc, pTmp, x1[:, tt, :], t0[:], bcast["ln1g"], bcast["ln1b"], epsT)
        X1T = big.tile([128, KT, SL], BF16, tag="XT")
        for tt in range(TT_L):
            for dp in range(KT):
                pt = psT.tile([128, 128], BF16, tag="pt")
                nc.tensor.transpose(pt[:], x1[:, tt, dp * 128:(dp + 1) * 128], identB[:])
                nc.vector.tensor_copy(X1T[:, dp, tt * 128:(tt + 1) * 128], pt[:])

        if stage == "E":
            pad = pers.tile([128, D], F32, tag="dbg")
            nc.vector.tensor_copy(pad[:], x1[:, 0, :])
            nc.sync.dma_start(out_d[:], pad[:])
            return

        # ---------------- Phase F: FFN + LN2 -> x2, X2T ----------------
        x2 = big.tile([128, TT_L, D], BF16, tag="OTn")
        for c2 in range(SL // 512):
            HT = big.tile([128, FT, 512], BF16, tag="sWqkv")
            for ft in range(FT):
                ps = psMM.tile([128, 512], F32, tag="mm")
                _mm_acc(nc, ps[:],
                        [W1_s[:, k, ft * 128:(ft + 1) * 128] for k in range(KT)],
                        [X1T[:, k, c2 * 512:(c2 + 1) * 512] for k in range(KT)])
                nc.scalar.activation(HT[:, ft, :], ps[:], AF.Relu,
                                     bias=b1_s[:, ft:ft + 1])
            for dp in range(KT):
                ps = psMM.tile([128, 512], F32, tag="mm")
                _mm_acc(nc, ps[:],
                        [W2_s[:, k, dp * 128:(dp + 1) * 128] for k in range(FT)],
                        [HT[:, k, :] for k in range(FT)])
                fft = pTmp.tile([128, 512], BF16, tag="fft")
                nc.vector.tensor_scalar_add(fft[:], in0=ps[:],
                                            scalar1=pp["b2"][:, dp:dp + 1])
                for st in range(4):
                    tt = c2 * 4 + st
                    pt = psT.tile([128, 128], BF16, tag="pt")
                    nc.tensor.transpose(pt[:], fft[:, st * 128:(st + 1) * 128], identB[:])
                    nc.vector.tensor_add(x2[:, tt, dp * 128:(dp + 1) * 128], pt[:],
                                         x1[:, tt, dp * 128:(dp + 1) * 128])
        X2T = big.tile([128, KT, SL], BF16, tag="QT")
        for tt in range(TT_L):
            _layernorm(nc, pTmp, x2[:, tt, :], x2[:, tt, :], bcast["ln2g"],
                       bcast["ln2b"], epsT)
            for dp in range(KT):
                pt = psT.tile([128, 128], BF16, tag="pt")
                nc.tensor.transpose(pt[:], x2[:, tt, dp * 128:(dp + 1) * 128], identB[:])
                nc.vector.tensor_copy(X2T[:, dp, tt * 128:(tt + 1) * 128], pt[:])
        # local x2 rows to DRAM for the patch-query gather
        nc.sync.dma_start(
            out=x2loc.bitcast(BF16).rearrange("(tt p d) -> p tt d", p=128, d=D),
            in_=x2[:])

        if stage == "F":
            pad = pers.tile([128, D], F32, tag="dbg")
            nc.vector.tensor_copy(pad[:], x2[:, 0, :])
            nc.sync.dma_start(out_d[:], pad[:])
            return

        # ---------------- Phase G: exchange patch-query rows ----------------
        qc_t = pers.tile([128, 2], I32)
        nc.sync.dma_start(qc_t[:], dt["qc_idx"][:])
        qcand = pTmp.tile([128, 2, D], BF16, tag="qcand", bufs=1)
        for j in range(2):
            nc.gpsimd.indirect_dma_start(
                out=qcand[:, j, :], out_offset=None,
                in_=x2loc.bitcast(BF16).rearrange("(t d) -> t d", d=D),
                in_offset=bass.IndirectOffsetOnAxis(ap=qc_t[:, j:j + 1], axis=0))
        nc.sync.dma_start(
            out=qx_in.bitcast(BF16).rearrange("(j p d) -> p j d", p=128, d=D),
            in_=qcand[:])
        nc.gpsimd.collective_compute(
            "AllGather", ALU.bypass, replica_groups=groups,
            ins=[qx_in.opt()], outs=[qx_all.opt()])

        # ---------------- Phase H: cross-attention (local keys) ----------------
        cKT = big.tile([128, KT, SL], BF16, tag="KTl")
        for dp in range(KT):
            for c2 in range(SL // 512):
                ps = psMM.tile([128, 512], F32, tag="mm")
                _mm_acc(nc, ps[:],
                        [cWk_s[:, k, dp * 128:(dp + 1) * 128] for k in range(KT)],
                        [X2T[:, k, c2 * 512:(c2 + 1) * 512] for k in range(KT)])
                nc.vector.tensor_scalar_add(cKT[:, dp, c2 * 512:(c2 + 1) * 512],
                                            in0=ps[:], scalar1=pp["cbk"][:, dp:dp + 1])
        cV1 = big.tile([128, TT_L, H, DH + 1], BF16, tag="V1l")
        nc.vector.memset(
            cV1[:, :, :, DH:DH + 1].rearrange("p a b c -> p (a b c)"), 1.0)
        for tt in range(TT_L):
            ps = psMM.tile([128, 512], F32, tag="mm")
            _mm_acc(nc, ps[:],
                    [X2T[:, k, tt * 128:(tt + 1) * 128] for k in range(KT)],
                    [cWv_s[:, k, :] for k in range(KT)])
            nc.vector.tensor_add(
                cV1[:, tt, :, 0:DH],
                ps[:].rearrange("p (h d) -> p h d", h=H),
                bcast["cbv"].rearrange("p (h d) -> p h d", h=H))

        # assemble the 256 query rows (own-half queries first), then cQ^T
        qa_t = pers.tile([128, 2], I32)
        nc.sync.dma_start(qa_t[:], dt["qa_idx"][:])
        qrows = pTmp.tile([128, 2, D], BF16, tag="qcand", bufs=1)
        for j in range(2):
            nc.gpsimd.indirect_dma_start(
                out=qrows[:, j, :], out_offset=None,
                in_=qx_all.bitcast(BF16).rearrange("a (t d) -> (a t) d", d=D),
                in_offset=bass.IndirectOffsetOnAxis(ap=qa_t[:, j:j + 1], axis=0))
        qT = pers.tile([128, KT, PR_COL], BF16, tag="qT")
        for j in range(2):
            for dp in range(KT):
                pt = psT.tile([128, 128], BF16, tag="pt")
                nc.tensor.transpose(pt[:], qrows[:, j, dp * 128:(dp + 1) * 128], identB[:])
                nc.vector.tensor_copy(qT[:, dp, j * 128:(j + 1) * 128], pt[:])
        cQT = pers.tile([128, KT, PR_COL], BF16, tag="cQT")
        for dp in range(KT):
            ps = psMM.tile([128, PR_COL], F32, tag="mm")
            _mm_acc(nc, ps[:],
                    [cWq_s[:, k, dp * 128:(dp + 1) * 128] for k in range(KT)],
                    [qT[:, k, :] for k in range(KT)])
            nc.vector.tensor_scalar_add(cQT[:, dp, :], in0=ps[:],
                                        scalar1=pp["cbq"][:, dp:dp + 1])

        if stage == "G":
            pad = pers.tile([128, D], F32, tag="dbg")
            nc.vector.memset(pad[:], 0.0)
            nc.vector.tensor_copy(pad[:, 0:PR_COL].rearrange("p (a b) -> p a b", a=2),
                                  qrows[:, :, 0:128])
            nc.sync.dma_start(out_d[:], pad[:])
            return

        # partial attention over local keys for all 256 batch queries
        prtS = big.tile([DH + 1, H, PR_COL], F32, tag="emb")
        for h in range(H):
            hp, hr = h // 2, (h % 2) * DH
            avt = psAV.tile([DH + 1, PR_COL], F32, tag="avt")
            for tg in range(2):
                psX = psMM.tile([128, 4, PR_COL], F32, tag="mm")
                for ti in range(4):
                    tkt = tg * 4 + ti
                    nc.tensor.matmul(
                        psX[:, ti, :],
                        lhsT=cKT[hr:hr + DH, hp, tkt * 128:(tkt + 1) * 128],
                        rhs=cQT[hr:hr + DH, hp, :], start=True, stop=True)
                eC = pExp.tile([128, 4, PR_COL], BF16, tag="expT")
                nc.scalar.activation(
                    eC[:].rearrange("p a b -> p (a b)"),
                    psX[:].rearrange("p a b -> p (a b)"), AF.Exp, scale=SCALE)
                for ti in range(4):
                    tkt = tg * 4 + ti
                    nc.tensor.matmul(
                        avt[:], lhsT=cV1[:, tkt, h, :], rhs=eC[:, ti, :],
                        start=(tkt == 0), stop=(tkt == TT_L - 1))
            nc.vector.tensor_copy(prtS[:, h, :], avt[:])
        # ship partials: DRAM rows (h*65 + r) of length 256
        nc.sync.dma_start(
            out=pr_in.rearrange("(h r q) -> r h q", h=H, r=DH + 1),
            in_=prtS[:])
        nc.gpsimd.collective_compute(
            "AllGather", ALU.bypass, replica_groups=groups,
            ins=[pr_in.opt()], outs=[pr_all.opt()])

        # ---------------- Phase I: combine partials, normalize, O-proj ----------------
        pr_t = pers.tile([128, H], I32)
        nc.sync.dma_start(pr_t[:], dt["pr_idx"][:])
        prP = big.tile([128, H, PR_COL], F32, tag="KTr")
        for hh in range(H):
            nc.gpsimd.indirect_dma_start(
                out=prP[:, hh, :], out_offset=None,
                in_=pr_all.rearrange("a (t q) -> (a t) q", q=PR_COL),
                in_offset=bass.IndirectOffsetOnAxis(ap=pr_t[:, hh:hh + 1], axis=0))
        # partner's block has its own queries first: my queries sit at
        # columns 128:256 of the partner block; mine at 0:128 of my block.
        cfin = big.tile([DH + 1, H, PL], F32, tag="denSt")
        nc.vector.tensor_add(cfin[:], prtS[:, :, 0:PL],
                             prP[0:DH + 1, :, PL:PR_COL])
        den8c = big.tile([8, PL], F32, tag="den8l")
        nc.gpsimd.dma_start(
            out=den8c[:],
            in_=cfin[DH:DH + 1, :, :].rearrange("p a b -> p (a b)"))
        rcp8c = big.tile([8, PL], F32, tag="den8r")
        nc.vector.reciprocal(rcp8c[:], den8c[:])
        OcT = pers.tile([128, KT, PL], BF16, tag="OcT")
        for h in range(H):
            hp, hr = h // 2, (h % 2) * DH
            rcpRow = pTmp.tile([1, SL], F32, tag="lnt0")
            nc.sync.dma_start(rcpRow[0:1, 0:PL], rcp8c[h:h + 1, :])
            psB = psT.tile([DH, PL], F32, tag="pt")
            nc.tensor.matmul(
                psB[:], lhsT=onesf.bitcast(F32R)[0:1, :],
                rhs=rcpRow.bitcast(F32R)[0:1, 0:PL], start=True, stop=True)
            nc.vector.tensor_mul(OcT[hr:hr + DH, hp, :], cfin[0:DH, h, :], psB[:])
        ps = psMM.tile([128, 512], F32, tag="mm")
        _mm_acc(nc, ps[:],
                [OcT[:, k, :] for k in range(KT)],
                [cWo_s[:, k, :] for k in range(KT)])
        outsb = pers.tile([128, D], F32, tag="outsb")
        nc.vector.tensor_add(outsb[:], ps[:], bcast["cbo"])
        nc.sync.dma_start(out_d[:], outsb[:])


def _layernorm(nc, pool, out_ap, in_ap, g_b, b_b, epsT):
    st = pool.tile([128, 6], F32, tag="ln_st")
    nc.vector.bn_stats(out=st[:], in_=in_ap)
    mv = pool.tile([128, 2], F32, tag="ln_mv")
    nc.vector.bn_aggr(out=mv[:], in_=st[:])
    sd = pool.tile([128, 1], F32, tag="ln_sd")
    nc.scalar.activation(sd[:], mv[:, 1:2], AF.Sqrt, bias=epsT[:])
    nc.vector.reciprocal(sd[:], sd[:])
    tmp = pool.tile([128, D], BF16, tag="ln_tmp")
    nc.vector.tensor_scalar(out=tmp[:], in0=in_ap, scalar1=mv[:, 0:1], scalar2=sd[:],
                            op0=ALU.subtract, op1=ALU.mult)
    nc.vector.tensor_mul(tmp[:], tmp[:], g_b[:])
    nc.vector.tensor_add(out_ap, tmp[:], b_b[:])


def _ngram_hashes(bytes_seq):
    """int64-wraparound n-gram hashes, mod V.  [B, S] -> [len(NGRAMS), B, S]"""
    b = bytes_seq.astype(np.int64)
    out = np.zeros((len(NGRAMS), b.shape[0], S), dtype=np.int64)
    for j, n in enumerate(NGRAMS):
        h = np.zeros_like(b)
        for k in range(n):
            shift = n - 1 - k
            mult = np.int64(256) ** k
            shifted = np.zeros_like(b)
            shifted[:, shift:] = b[:, : S - shift]
            h = h + shifted * mult
        h = np.where(np.arange(S)[None, :] >= (n - 1), h, 0)
        out[j] = h % V
    return out


_PROGRAM = None


def _get_program():
    global _PROGRAM
    if _PROGRAM is None:
        _PROGRAM = _build_program()
    return _PROGRAM


def _bf16(x):
    import ml_dtypes
    return np.asarray(x, dtype=np.float32).astype(ml_dtypes.bfloat16)


def make_in_maps(inputs):
    bytes_seq = np.asarray(inputs["bytes_seq"])
    patch_idx = np.asarray(inputs["patch_idx"])
    byte_emb = np.asarray(inputs["byte_emb"], dtype=np.float32)
    ngram_emb = np.asarray(inputs["ngram_emb"], dtype=np.float32)

    table = _bf16(np.concatenate(
        [byte_emb, ngram_emb.reshape(len(NGRAMS) * V, D)], axis=0))
    hashes = _ngram_hashes(bytes_seq)

    weights = {}
    for w in _W512 + ["W1", "W2"]:
        weights[w] = np.ascontiguousarray(_bf16(inputs[w]))
    for bv in ["sbq", "sbk", "cbq", "cbk", "b2", "b1"]:
        weights[bv] = np.ascontiguousarray(np.asarray(inputs[bv], dtype=np.float32))
    for bv in ["sbv", "sbo", "ln1g", "ln1b", "ln2g", "ln2b", "cbv", "cbo"]:
        weights[bv] = np.ascontiguousarray(_bf16(inputs[bv]))

    in_maps = []
    for c in range(N_CORES):
        b, hh = c // 2, c % 2
        tok0 = hh * SL
        p_ar = np.arange(128)[:, None]
        tt_ar = np.arange(TT_L)[None, :]
        tok = tok0 + tt_ar * 128 + p_ar          # [128, TT_L]
        idx = np.zeros((128, NT, TT_L), dtype=np.int32)
        idx[:, 0, :] = bytes_seq[b][tok].astype(np.int32)
        for j in range(len(NGRAMS)):
            idx[:, 1 + j, :] = (256 + j * V + hashes[j, b][tok]).astype(np.int32)

        # remote rank's rows in the kv blobs
        kr_idx = ((1 - hh) * 128 + np.arange(128)).astype(np.int32)[:, None]

        # patch-query routing for this batch (both halves' info is shared)
        g = patch_idx[b].astype(np.int64)        # [256] global patch tokens
        half_of = (g // SL).astype(np.int32)     # which rank owns each row
        slot = np.zeros(P, dtype=np.int32)       # slot in that rank's qcand
        cnt = [0, 0]
        for q in range(P):
            slot[q] = cnt[half_of[q]]
            cnt[half_of[q]] += 1
        # qc_idx: rows of local x2 this core ships (its residents, in slot order)
        qc = np.zeros(PR_COL, dtype=np.int32)
        for q in range(P):
            if half_of[q] == hh:
                qc[slot[q]] = g[q] % SL
        qc_idx = qc.reshape(2, 128).T.copy()     # [128, 2] (j-major columns)
        # qa_idx: assemble 256 rows own-half-queries-first from qx_all
        qa = np.zeros(PR_COL, dtype=np.int32)
        for i in range(PL):
            for j in range(2):
                q = (hh if j == 0 else 1 - hh) * PL + i
                qa[j * PL + i] = half_of[q] * PR_COL + slot[q]
        qa_idx = qa.reshape(2, 128).T.copy()
        # pr_idx: partner partial rows (h*65 + r) for r<65, else row 0
        pr = np.zeros((128, H), dtype=np.int32)
        for r in range(128):
            for h2 in range(H):
                pr[r, h2] = (1 - hh) * PR_ROWS + h2 * (DH + 1) + r if r <= DH else 0
        m = {"table": table, "idx": idx,
             "kr_idx": kr_idx.astype(np.int32),
             "qc_idx": np.ascontiguousarray(qc_idx),
             "qa_idx": np.ascontiguousarray(qa_idx),
             "pr_idx": pr}
        m.update(weights)
        in_maps.append(m)
    return in_maps


def assemble_output(results):
    out = np.zeros((B, P, D), dtype=np.float32)
    for c in range(N_CORES):
        b, hh = c // 2, c % 2
        out[b, hh * PL:(hh + 1) * PL, :] = results[c]["out"]
    return out


def kernel(**inputs):
    nc = _get_program()
    in_maps = make_in_maps(inputs)
    res = run_bass_kernel_spmd(nc, in_maps, core_ids=list(range(N_CORES)))
    return assemble_output(res.results)


if __name__ == "__main__":
    _build_program()
    print("program built OK")


# revision 3
# speedup vs baseline: 3.3211x; 1.0318x over previous
"""Trainium2 Bass kernel for nn_ByteEncoder (optimized v2).

Model: byte + 6 n-gram hash embeddings summed -> one post-norm transformer
encoder layer (MHA + relu FFN) -> cross-attention from patch-boundary queries
to the full sequence.

Sharding: 8 cores; core c handles batch b=c//2, sequence half h=c%2 (1024
tokens).  All matmul operands bf16 (f32 PSUM accumulate).  The embedding
tables are gathered with the SDMA inline adder (7 accumulating indirect
DMAs).  Self-attention runs local keys first so the pairwise K/V AllGather
overlaps with compute; the attention A@V is computed feature-major
(out = V'^T E) with a ones-column producing the softmax denominators.
Cross-attention is key-split across the pair: only the 256 patch-query rows
(256KB) and the partial attention accumulators (532KB) are exchanged instead
of the full x2 activations.
"""

import sys
import numpy as np

sys.path.insert(0, "/opt/trn_rl_repo")

import concourse.bass as bass
import concourse.bacc as bacc
import concourse.tile as tile
import concourse.mybir as mybir
from concourse.bass_utils import run_bass_kernel_spmd
from concourse.masks import make_identity

F32 = mybir.dt.float32
F32R = mybir.dt.float32r
BF16 = mybir.dt.bfloat16
I32 = mybir.dt.int32
AF = mybir.ActivationFunctionType
ALU = mybir.AluOpType

B, S, D, H, V, P = 4, 2048, 512, 8, 100000, 256
NGRAMS = list(range(3, 9))
NT = 1 + len(NGRAMS)          # 7 tables (byte + 6 ngram)
DH = D // H                   # 64
DF = 4 * D                    # 2048
SCALE = float(np.float32(DH) ** -0.5)
N_CORES = 8
SL = S // 2                   # 1024 local tokens
PL = P // 2                   # 128 local queries
KT = D // 128                 # 4 feature blocks
TT_L = SL // 128              # 8 local token tiles
FT = DF // 128                # 16 tiles over d_ff
VROWS = 256 + len(NGRAMS) * V

# DRAM exchange blob geometry (all byte-counted as f32 elements for the
# collective tensors; bf16 payload is bitcast)
K_ROW = KT * SL               # 4096 bf16 per partition-row
V_ROW = TT_L * H * (DH + 1)   # 4160 bf16 per partition-row
Q_ROW = D                     # 512 bf16 per gathered query row
PR_ROWS = H * (DH + 1)        # 520 f32 rows in the partial blob
PR_COL = 2 * PL               # 256 queries per batch

_W512 = ["sWq", "sWk", "sWv", "sWo", "cWq", "cWk", "cWv", "cWo"]


def _build_program(stage="Z"):
    nc = bacc.Bacc("TRN2", target_bir_lowering=False, debug=False,
                   num_devices=N_CORES)
    dt = {}
    dt["table"] = nc.dram_tensor("table", [VROWS, D], BF16, kind="ExternalInput").ap()
    dt["idx"] = nc.dram_tensor("idx", [128, TT_L, NT], I32, kind="ExternalInput").ap()
    dt["kr_idx"] = nc.dram_tensor("kr_idx", [128, 1], I32, kind="ExternalInput").ap()
    dt["qc_idx"] = nc.dram_tensor("qc_idx", [128, 2], I32, kind="ExternalInput").ap()
    dt["qa_idx"] = nc.dram_tensor("qa_idx", [128, 2], I32, kind="ExternalInput").ap()
    dt["pr_idx"] = nc.dram_tensor("pr_idx", [128, H], I32, kind="ExternalInput").ap()
    for w in _W512:
        dt[w] = nc.dram_tensor(w, [D, D], BF16, kind="ExternalInput").ap()
    dt["W1"] = nc.dram_tensor("W1", [D, DF], BF16, kind="ExternalInput").ap()
    dt["W2"] = nc.dram_tensor("W2", [DF, D], BF16, kind="ExternalInput").ap()
    # per-partition (feature-major) f32 biases
    for bv in ["sbq", "sbk", "cbq", "cbk", "b2"]:
        dt[bv] = nc.dram_tensor(bv, [D], F32, kind="ExternalInput").ap()
    dt["b1"] = nc.dram_tensor("b1", [DF], F32, kind="ExternalInput").ap()
    # broadcast-row bf16 biases
    for bv in ["sbv", "sbo", "ln1g", "ln1b", "ln2g", "ln2b", "cbv", "cbo"]:
        dt[bv] = nc.dram_tensor(bv, [D], BF16, kind="ExternalInput").ap()
    out_d = nc.dram_tensor("out", [PL, D], F32, kind="ExternalOutput").ap()

    # DRAM bounce buffers for the pair collectives (f32-typed, bf16 payload)
    kv_in_k = nc.dram_tensor("kv_in_k", [128 * K_ROW // 2], F32, kind="Internal").ap()
    kv_all_k = nc.dram_tensor("kv_all_k", [2, 128 * K_ROW // 2], F32, kind="Internal").ap()
    kv_in_v = nc.dram_tensor("kv_in_v", [128 * V_ROW // 2], F32, kind="Internal").ap()
    kv_all_v = nc.dram_tensor("kv_all_v", [2, 128 * V_ROW // 2], F32, kind="Internal").ap()
    x2loc = nc.dram_tensor("x2loc", [SL * D // 2], F32, kind="Internal").ap()
    qx_in = nc.dram_tensor("qx_in", [PR_COL * Q_ROW // 2], F32, kind="Internal").ap()
    qx_all = nc.dram_tensor("qx_all", [2, PR_COL * Q_ROW // 2], F32, kind="Internal").ap()
    pr_in = [nc.dram_tensor(f"pr_in{i}", [PR_ROWS // 2 * PR_COL], F32, kind="Internal").ap() for i in range(2)]
    pr_all = [nc.dram_tensor(f"pr_all{i}", [2, PR_ROWS // 2 * PR_COL], F32, kind="Internal").ap() for i in range(2)]
    groups = [[0, 1], [2, 3], [4, 5], [6, 7]]

    with tile.TileContext(nc) as tc:
        _emit(nc, tc, dt, out_d, kv_in_k, kv_all_k, kv_in_v, kv_all_v,
              x2loc, qx_in, qx_all, pr_in, pr_all, groups, stage)
    nc.compile()
    return nc


def _mm_acc(nc, ps, lhsT_tiles, rhs_tiles):
    n = len(lhsT_tiles)
    for k in range(n):
        nc.tensor.matmul(ps, lhsT=lhsT_tiles[k], rhs=rhs_tiles[k],
                         start=(k == 0), stop=(k == n - 1))


def _emit(nc, tc, dt, out_d, kv_in_k, kv_all_k, kv_in_v, kv_all_v,
          x2loc, qx_in, qx_all, pr_in, pr_all, groups, stage="Z"):
    from contextlib import ExitStack

    ctx = ExitStack()
    with ctx:
        big = ctx.enter_context(tc.tile_pool(name="big", bufs=1))
        pers = ctx.enter_context(tc.tile_pool(name="pers", bufs=1))
        pExp = ctx.enter_context(tc.tile_pool(name="pExp", bufs=3))
        pTmp = ctx.enter_context(tc.tile_pool(name="pTmp", bufs=2))
        psMM = ctx.enter_context(tc.tile_pool(name="psMM", bufs=2, space="PSUM"))
        psAV = ctx.enter_context(tc.tile_pool(name="psAV", bufs=1, space="PSUM"))
        psT = ctx.enter_context(tc.tile_pool(name="psT", bufs=2, space="PSUM"))

        identB = pers.tile([128, 128], BF16)
        make_identity(nc, identB[:])
        epsT = pers.tile([128, 1], F32)
        nc.vector.memset(epsT[:], 1e-5)
        onesb = pers.tile([128, DH], BF16)
        nc.vector.memset(onesb[:], 1.0)
        onesf = pers.tile([128, DH], F32)
        nc.vector.memset(onesf[:], 1.0)

        # broadcast-along-partition bias rows (bf16)
        def load_bcast(tile_, i, name):
            src = dt[name]
            bc_ap = bass.AP(tensor=src.tensor, offset=src.offset,
                            ap=[[0, 128]] + list(src.ap))
            nc.gpsimd.dma_start(out=tile_[:, i, :], in_=bc_ap)
            return tile_[:, i, :]

        bc1 = pers.tile([128, 8, D], BF16, tag="bc")
        bcast = {}
        # per-partition (feature-major) f32 bias tiles
        pp = {}
        for name in ["sbq", "sbk", "cbq", "cbk", "b2"]:
            t = pers.tile([128, KT], F32, tag=f"pp_{name}")
            nc.sync.dma_start(out=t[:], in_=dt[name].rearrange("(dp p) -> p dp", p=128))
            pp[name] = t
        b1_s = pers.tile([128, FT], F32)
        nc.sync.dma_start(out=b1_s[:], in_=dt["b1"].rearrange("(dp p) -> p dp", p=128))

        # self-attn QKV weights, feature-major bf16
        sWqkv = big.tile([128, 3, KT, D], BF16, tag="sWqkv")
        for i, name in enumerate(["sWq", "sWk", "sWv"]):
            nc.sync.dma_start(
                out=sWqkv[:, i, :, :],
                in_=dt[name].rearrange("(kt p) n -> p kt n", p=128))
        sWq_s, sWk_s, sWv_s = sWqkv[:, 0], sWqkv[:, 1], sWqkv[:, 2]

        # ---------------- Phase A: batched gather + emb ----------------
        idx_t = pers.tile([128, TT_L, NT], I32)
        nc.sync.dma_start(idx_t[:], dt["idx"][:])
        emb = big.tile([128, TT_L, D], BF16, tag="emb")
        for tt in range(TT_L):
            e7 = big.tile([128, NT, D], BF16, tag=("KTr" if tt % 2 == 0 else "V1r"), bufs=1)
            for j in range(NT):
                gi = nc.gpsimd.indirect_dma_start(
                    out=e7[:, j, :], out_offset=None, in_=dt["table"][:],
                    in_offset=bass.IndirectOffsetOnAxis(
                        ap=idx_t[:, tt, j:j + 1], axis=0))
            a01 = pTmp.tile([128, D], BF16, tag="ga")
            nc.vector.tensor_add(a01[:], e7[:, 0, :], e7[:, 1, :])
            a23 = pTmp.tile([128, D], BF16, tag="gb")
            nc.vector.tensor_add(a23[:], e7[:, 2, :], e7[:, 3, :])
            a45 = pTmp.tile([128, D], BF16, tag="gc")
            nc.vector.tensor_add(a45[:], e7[:, 4, :], e7[:, 5, :])
            nc.vector.tensor_add(a01[:], a01[:], e7[:, 6, :])
            nc.vector.tensor_add(a23[:], a23[:], a45[:])
            nc.vector.tensor_add(a01[:], a01[:], a23[:])
            nc.vector.tensor_scalar_mul(emb[:, tt, :], in0=a01[:],
                                        scalar1=1.0 / NT)

        if stage == "A":
            pad = pers.tile([128, D], F32, tag="dbg")
            nc.vector.tensor_copy(pad[:], emb[:, 0, :])
            nc.sync.dma_start(out_d[:], pad[:])
            return
        for i, name in enumerate(["sbv", "sbo", "ln1g", "ln1b",
                                  "ln2g", "ln2b", "cbv", "cbo"]):
            bcast[name] = load_bcast(bc1, i, name)
        # X^T feature-major bf16
        XT = big.tile([128, KT, SL], BF16, tag="XT")
        for tt in range(TT_L):
            for dp in range(KT):
                pt = psT.tile([128, 128], BF16, tag="pt")
                nc.tensor.transpose(pt[:], emb[:, tt, dp * 128:(dp + 1) * 128], identB[:])
                nc.vector.tensor_copy(XT[:, dp, tt * 128:(tt + 1) * 128], pt[:])

        # ---------------- Phase B: K,V then exchange, then Q ----------------
        KTl = big.tile([128, KT, SL], BF16, tag="KTl")
        for dp in range(KT):
            for c2 in range(SL // 512):
                ps = psMM.tile([128, 512], F32, tag="mm")
                _mm_acc(nc, ps[:],
                        [sWk_s[:, k, dp * 128:(dp + 1) * 128] for k in range(KT)],
                        [XT[:, k, c2 * 512:(c2 + 1) * 512] for k in range(KT)])
                nc.vector.tensor_scalar_add(KTl[:, dp, c2 * 512:(c2 + 1) * 512],
                                            in0=ps[:], scalar1=pp["sbk"][:, dp:dp + 1])
        V1l = big.tile([128, TT_L, H, DH + 1], BF16, tag="V1l")
        nc.vector.memset(
            V1l[:, :, :, DH:DH + 1].rearrange("p a b c -> p (a b c)"), 1.0)
        for tt in range(TT_L):
            ps = psMM.tile([128, 512], F32, tag="mm")
            _mm_acc(nc, ps[:],
                    [XT[:, k, tt * 128:(tt + 1) * 128] for k in range(KT)],
                    [sWv_s[:, k, :] for k in range(KT)])
            nc.vector.tensor_add(
                V1l[:, tt, :, 0:DH],
                ps[:].rearrange("p (h d) -> p h d", h=H),
                bcast["sbv"].rearrange("p (h d) -> p h d", h=H))

        # ship local K^T and V' to the partner (two blobs, plain row
        # gathers on the far side -- element_offset gathers measured slow)
        nc.sync.dma_start(
            out=kv_in_k.bitcast(BF16).rearrange("(p x) -> p x", p=128),
            in_=KTl[:].rearrange("p a b -> p (a b)"))
        nc.sync.dma_start(
            out=kv_in_v.bitcast(BF16).rearrange("(p x) -> p x", p=128),
            in_=V1l[:].rearrange("p a b c -> p (a b c)"))
        nc.gpsimd.collective_compute(
            "AllGather", ALU.bypass, replica_groups=groups,
            ins=[kv_in_k.opt()], outs=[kv_all_k.opt()])
        nc.gpsimd.collective_compute(
            "AllGather", ALU.bypass, replica_groups=groups,
            ins=[kv_in_v.opt()], outs=[kv_all_v.opt()])

        QT = big.tile([128, KT, SL], BF16, tag="QT")
        for dp in range(KT):
            for c2 in range(SL // 512):
                ps = psMM.tile([128, 512], F32, tag="mm")
                _mm_acc(nc, ps[:],
                        [sWq_s[:, k, dp * 128:(dp + 1) * 128] for k in range(KT)],
                        [XT[:, k, c2 * 512:(c2 + 1) * 512] for k in range(KT)])
                nc.vector.tensor_scalar_add(QT[:, dp, c2 * 512:(c2 + 1) * 512],
                                            in0=ps[:], scalar1=pp["sbq"][:, dp:dp + 1])

        if stage == "B":
            pad = pers.tile([128, D], F32, tag="dbg")
            nc.vector.tensor_copy(pad[:].rearrange("p (a b) -> p a b", a=KT),
                                  QT[:, :, 0:128])
            nc.sync.dma_start(out_d[:], pad[:])
            return

        # prefetch heavy later-phase weights while attention runs
        sWo_s = big.tile([128, KT, D], BF16, tag="sWo")
        nc.sync.dma_start(out=sWo_s[:],
                          in_=dt["sWo"].rearrange("(kt p) n -> p kt n", p=128))
        W1_s = big.tile([128, KT, DF], BF16, tag="W1")
        nc.sync.dma_start(out=W1_s[:],
                          in_=dt["W1"].rearrange("(kt p) n -> p kt n", p=128))
        W2_s = big.tile([128, FT, D], BF16, tag="W2")
        nc.sync.dma_start(out=W2_s[:],
                          in_=dt["W2"].rearrange("(kt p) n -> p kt n", p=128))
        # ---------------- Phase C: self-attention, local keys ----------------
        # OTP: unnormalized sum(exp*V)^T partials, f32. den8: denominators.
        OTP = big.tile([128, KT, SL], BF16, tag="OTP")
        denSt = big.tile([65, SL], F32, tag="denSt")
        den8l = big.tile([8, SL], F32, tag="den8l")
        den8r = big.tile([8, SL], F32, tag="den8r")

        def attn_half(KTx, V1x, local):
            for h in range(H):
                hp, hr = h // 2, (h % 2) * DH
                avt = psAV.tile([DH + 1, SL], F32, tag="avt")
                for tkt in range(TT_L):
                    psS = psMM.tile([128, SL], F32, tag="mm")
                    for j in range(SL // 512):
                        nc.tensor.matmul(
                            psS[:, j * 512:(j + 1) * 512],
                            lhsT=KTx[hr:hr + DH, hp, tkt * 128:(tkt + 1) * 128],
                            rhs=QT[hr:hr + DH, hp, j * 512:(j + 1) * 512],
                            start=True, stop=True)
                    eT = pExp.tile([128, SL], BF16, tag="expT")
                    nc.scalar.activation(eT[:], psS[:], AF.Exp, scale=SCALE)
                    for j in range(SL // 512):
                        nc.tensor.matmul(
                            avt[:, j * 512:(j + 1) * 512],
                            lhsT=V1x[:, tkt, h, :],
                            rhs=eT[:, j * 512:(j + 1) * 512],
                            start=(tkt == 0), stop=(tkt == TT_L - 1))
                if local:
                    nc.vector.tensor_copy(OTP[hr:hr + DH, hp, :], avt[0:DH, :])
                    nc.vector.tensor_copy(denSt[DH:DH + 1, :], avt[DH:DH + 1, :])
                    nc.sync.dma_start(den8l[h:h + 1, :], denSt[DH:DH + 1, :])
                else:
                    nc.vector.tensor_add(OTP[hr:hr + DH, hp, :],
                                         avt[0:DH, :], OTP[hr:hr + DH, hp, :])
                    nc.vector.tensor_copy(denSt[DH:DH + 1, :], avt[DH:DH + 1, :])
                    nc.sync.dma_start(den8r[h:h + 1, :], denSt[DH:DH + 1, :])

        attn_half(KTl, V1l, True)

        # ---------------- Phase D: remote keys ----------------
        kr_t = pers.tile([128, 1], I32)
        nc.sync.dma_start(kr_t[:], dt["kr_idx"][:])
        KTr = big.tile([128, KT, SL], BF16, tag="KTr")
        nc.gpsimd.indirect_dma_start(
            out=KTr[:].rearrange("p a b -> p (a b)"), out_offset=None,
            in_=kv_all_k.bitcast(BF16).rearrange("a (b x) -> (a b) x", x=K_ROW),
            in_offset=bass.IndirectOffsetOnAxis(ap=kr_t[:, 0:1], axis=0))
        V1r = big.tile([128, TT_L, H, DH + 1], BF16, tag="V1r")
        nc.gpsimd.indirect_dma_start(
            out=V1r[:].rearrange("p a b c -> p (a b c)"), out_offset=None,
            in_=kv_all_v.bitcast(BF16).rearrange("a (b x) -> (a b) x", x=V_ROW),
            in_offset=bass.IndirectOffsetOnAxis(ap=kr_t[:, 0:1], axis=0))

        attn_half(KTr, V1r, False)

        # normalization: OTn = OTP * (1/den) broadcast along features
        den8 = big.tile([8, SL], F32, tag="denSt", bufs=1)
        nc.vector.tensor_add(den8[:], den8l[:], den8r[:])
        rcp8 = big.tile([8, SL], F32, tag="den8r", bufs=1)
        nc.vector.reciprocal_approx_fast(rcp8[:], den8[:])
        OTn = big.tile([128, KT, SL], BF16, tag="OTn")
        for h in range(H):
            hp, hr = h // 2, (h % 2) * DH
            rcpRow = pTmp.tile([1, SL], F32, tag="lnt0")
            nc.sync.dma_start(rcpRow[:], rcp8[h:h + 1, :])
            for j in range(SL // 512):
                psB = psT.tile([DH, 512], F32, tag="pt")
                nc.tensor.matmul(
                    psB[:], lhsT=onesf.bitcast(F32R)[0:1, :],
                    rhs=rcpRow.bitcast(F32R)[0:1, j * 512:(j + 1) * 512],
                    start=True, stop=True)
                nc.vector.tensor_mul(OTn[hr:hr + DH, hp, j * 512:(j + 1) * 512],
                                     OTP[hr:hr + DH, hp, j * 512:(j + 1) * 512],
                                     psB[:])

        if stage == "C":
            pad = pers.tile([128, D], F32, tag="dbg")
            nc.vector.tensor_copy(pad[:].rearrange("p (a b) -> p a b", a=KT),
                                  OTn[:, :, 0:128])
            nc.sync.dma_start(out_d[:], pad[:])
            return

        # cross-attn weights reuse the (now dead) OTP slot
        cWall = big.tile([128, 4, KT, D], BF16, tag="OTP")
        for i, name in enumerate(["cWq", "cWk", "cWv", "cWo"]):
            nc.sync.dma_start(
                out=cWall[:, i, :, :],
                in_=dt[name].rearrange("(kt p) n -> p kt n", p=128))
        cWq_s, cWk_s, cWv_s, cWo_s = (cWall[:, i] for i in range(4))

        # ---------------- Phase E: O-proj, +emb, LN1 ----------------
        x1 = big.tile([128, TT_L, D], BF16, tag="x1")
        for tt in range(TT_L):
            ps = psMM.tile([128, 512], F32, tag="mm")
            _mm_acc(nc, ps[:],
                    [OTn[:, k, tt * 128:(tt + 1) * 128] for k in range(KT)],
                    [sWo_s[:, k, :] for k in range(KT)])
            t0 = pTmp.tile([128, D], F32, tag="lnt0")
            nc.vector.tensor_add(t0[:], ps[:], bcast["sbo"])
            nc.vector.tensor_add(t0[:], t0[:], emb[:, tt, :])
            _layernorm(nc, pTmp, x1[:, tt, :], t0[:], bcast["ln1g"], bcast["ln1b"], epsT)
        X1T = big.tile([128, KT, SL], BF16, tag="XT")
        for tt in range(TT_L):
            for dp in range(KT):
                pt = psT.tile([128, 128], BF16, tag="pt")
                nc.tensor.transpose(pt[:], x1[:, tt, dp * 128:(dp + 1) * 128], identB[:])
                nc.scalar.copy(X1T[:, dp, tt * 128:(tt + 1) * 128], pt[:])

        if stage == "E":
            pad = pers.tile([128, D], F32, tag="dbg")
            nc.vector.tensor_copy(pad[:], x1[:, 0, :])
            nc.sync.dma_start(out_d[:], pad[:])
            return

        # ---------------- Phase F: FFN + LN2 -> x2, X2T ----------------
        x2 = big.tile([128, TT_L, D], BF16, tag="OTn")
        for c2 in range(SL // 512):
            HT = big.tile([128, FT, 512], BF16, tag="sWqkv")
            for ft in range(FT):
                ps = psMM.tile([128, 512], F32, tag="mm")
                _mm_acc(nc, ps[:],
                        [W1_s[:, k, ft * 128:(ft + 1) * 128] for k in range(KT)],
                        [X1T[:, k, c2 * 512:(c2 + 1) * 512] for k in range(KT)])
                nc.scalar.activation(HT[:, ft, :], ps[:], AF.Relu,
                                     bias=b1_s[:, ft:ft + 1])
            for dp in range(KT):
                ps = psMM.tile([128, 512], F32, tag="mm")
                _mm_acc(nc, ps[:],
                        [W2_s[:, k, dp * 128:(dp + 1) * 128] for k in range(FT)],
                        [HT[:, k, :] for k in range(FT)])
                fft = pTmp.tile([128, 512], BF16, tag="fft")
                nc.vector.tensor_scalar_add(fft[:], in0=ps[:],
                                            scalar1=pp["b2"][:, dp:dp + 1])
                for st in range(4):
                    tt = c2 * 4 + st
                    pt = psT.tile([128, 128], BF16, tag="pt")
                    nc.tensor.transpose(pt[:], fft[:, st * 128:(st + 1) * 128], identB[:])
                    nc.vector.tensor_add(x2[:, tt, dp * 128:(dp + 1) * 128], pt[:],
                                         x1[:, tt, dp * 128:(dp + 1) * 128])
        X2T = big.tile([128, KT, SL], BF16, tag="QT")
        for tt in range(TT_L):
            _layernorm(nc, pTmp, x2[:, tt, :], x2[:, tt, :], bcast["ln2g"],
                       bcast["ln2b"], epsT)
            for dp in range(KT):
                pt = psT.tile([128, 128], BF16, tag="pt")
                nc.tensor.transpose(pt[:], x2[:, tt, dp * 128:(dp + 1) * 128], identB[:])
                nc.scalar.copy(X2T[:, dp, tt * 128:(tt + 1) * 128], pt[:])
        # local x2 rows to DRAM for the patch-query gather
        nc.sync.dma_start(
            out=x2loc.bitcast(BF16).rearrange("(tt p d) -> p tt d", p=128, d=D),
            in_=x2[:])

        if stage == "F":
            pad = pers.tile([128, D], F32, tag="dbg")
            nc.vector.tensor_copy(pad[:], x2[:, 0, :])
            nc.sync.dma_start(out_d[:], pad[:])
            return

        # ---------------- Phase G: exchange patch-query rows ----------------
        qc_t = pers.tile([128, 2], I32)
        nc.sync.dma_start(qc_t[:], dt["qc_idx"][:])
        qcand = pTmp.tile([128, 2, D], BF16, tag="qcand", bufs=1)
        for j in range(2):
            nc.gpsimd.indirect_dma_start(
                out=qcand[:, j, :], out_offset=None,
                in_=x2loc.bitcast(BF16).rearrange("(t d) -> t d", d=D),
                in_offset=bass.IndirectOffsetOnAxis(ap=qc_t[:, j:j + 1], axis=0))
        nc.sync.dma_start(
            out=qx_in.bitcast(BF16).rearrange("(j p d) -> p j d", p=128, d=D),
            in_=qcand[:])
        nc.gpsimd.collective_compute(
            "AllGather", ALU.bypass, replica_groups=groups,
            ins=[qx_in.opt()], outs=[qx_all.opt()])

        # ---------------- Phase H: cross-attention (local keys) ----------------
        cKT = big.tile([128, KT, SL], BF16, tag="KTl")
        for dp in range(KT):
            for c2 in range(SL // 512):
                ps = psMM.tile([128, 512], F32, tag="mm")
                _mm_acc(nc, ps[:],
                        [cWk_s[:, k, dp * 128:(dp + 1) * 128] for k in range(KT)],
                        [X2T[:, k, c2 * 512:(c2 + 1) * 512] for k in range(KT)])
                nc.vector.tensor_scalar_add(cKT[:, dp, c2 * 512:(c2 + 1) * 512],
                                            in0=ps[:], scalar1=pp["cbk"][:, dp:dp + 1])
        cV1 = big.tile([128, TT_L, H, DH + 1], BF16, tag="V1l")
        nc.vector.memset(
            cV1[:, :, :, DH:DH + 1].rearrange("p a b c -> p (a b c)"), 1.0)
        for tt in range(TT_L):
            ps = psMM.tile([128, 512], F32, tag="mm")
            _mm_acc(nc, ps[:],
                    [X2T[:, k, tt * 128:(tt + 1) * 128] for k in range(KT)],
                    [cWv_s[:, k, :] for k in range(KT)])
            nc.vector.tensor_add(
                cV1[:, tt, :, 0:DH],
                ps[:].rearrange("p (h d) -> p h d", h=H),
                bcast["cbv"].rearrange("p (h d) -> p h d", h=H))

        # assemble the 256 query rows (own-half queries first), then cQ^T
        qa_t = pers.tile([128, 2], I32)
        nc.sync.dma_start(qa_t[:], dt["qa_idx"][:])
        qrows = pTmp.tile([128, 2, D], BF16, tag="qcand", bufs=1)
        for j in range(2):
            nc.gpsimd.indirect_dma_start(
                out=qrows[:, j, :], out_offset=None,
                in_=qx_all.bitcast(BF16).rearrange("a (t d) -> (a t) d", d=D),
                in_offset=bass.IndirectOffsetOnAxis(ap=qa_t[:, j:j + 1], axis=0))
        qT = pers.tile([128, KT, PR_COL], BF16, tag="qT")
        for j in range(2):
            for dp in range(KT):
                pt = psT.tile([128, 128], BF16, tag="pt")
                nc.tensor.transpose(pt[:], qrows[:, j, dp * 128:(dp + 1) * 128], identB[:])
                nc.vector.tensor_copy(qT[:, dp, j * 128:(j + 1) * 128], pt[:])
        cQT = pers.tile([128, KT, PR_COL], BF16, tag="cQT")
        for dp in range(KT):
            ps = psMM.tile([128, PR_COL], F32, tag="mm")
            _mm_acc(nc, ps[:],
                    [cWq_s[:, k, dp * 128:(dp + 1) * 128] for k in range(KT)],
                    [qT[:, k, :] for k in range(KT)])
            nc.vector.tensor_scalar_add(cQT[:, dp, :], in0=ps[:],
                                        scalar1=pp["cbq"][:, dp:dp + 1])

        if stage == "G":
            pad = pers.tile([128, D], F32, tag="dbg")
            nc.vector.memset(pad[:], 0.0)
            nc.vector.tensor_copy(pad[:, 0:PR_COL].rearrange("p (a b) -> p a b", a=2),
                                  qrows[:, :, 0:128])
            nc.sync.dma_start(out_d[:], pad[:])
            return

        # partial attention over local keys for all 256 batch queries
        prtS = big.tile([DH + 1, H, PR_COL], F32, tag="emb")
        for h in range(H):
            hp, hr = h // 2, (h % 2) * DH
            avt = psAV.tile([DH + 1, PR_COL], F32, tag="avt")
            for tg in range(2):
                psX = psMM.tile([128, 4, PR_COL], F32, tag="mm")
                for ti in range(4):
                    tkt = tg * 4 + ti
                    nc.tensor.matmul(
                        psX[:, ti, :],
                        lhsT=cKT[hr:hr + DH, hp, tkt * 128:(tkt + 1) * 128],
                        rhs=cQT[hr:hr + DH, hp, :], start=True, stop=True)
                eC = pExp.tile([128, 4, PR_COL], BF16, tag="expT")
                nc.scalar.activation(
                    eC[:].rearrange("p a b -> p (a b)"),
                    psX[:].rearrange("p a b -> p (a b)"), AF.Exp, scale=SCALE)
                for ti in range(4):
                    tkt = tg * 4 + ti
                    nc.tensor.matmul(
                        avt[:], lhsT=cV1[:, tkt, h, :], rhs=eC[:, ti, :],
                        start=(tkt == 0), stop=(tkt == TT_L - 1))
            nc.vector.tensor_copy(prtS[:, h, :], avt[:])
            if h == 3 or h == 7:
                g2 = h // 4
                nc.sync.dma_start(
                    out=pr_in[g2].rearrange("(h r q) -> r h q", h=H // 2, r=DH + 1),
                    in_=prtS[:, g2 * 4:(g2 + 1) * 4, :])
                nc.gpsimd.collective_compute(
                    "AllGather", ALU.bypass, replica_groups=groups,
                    ins=[pr_in[g2].opt()], outs=[pr_all[g2].opt()])

        # ---------------- Phase I: combine partials, normalize, O-proj ----------------
        pr_t = pers.tile([128, H], I32)
        nc.sync.dma_start(pr_t[:], dt["pr_idx"][:])
        prP = big.tile([128, H, PR_COL], F32, tag="KTr")
        for hh in range(H):
            nc.gpsimd.indirect_dma_start(
                out=prP[:, hh, :], out_offset=None,
                in_=pr_all[hh // 4].rearrange("a (t q) -> (a t) q", q=PR_COL),
                in_offset=bass.IndirectOffsetOnAxis(ap=pr_t[:, hh:hh + 1], axis=0))
        # partner's block has its own queries first: my queries sit at
        # columns 128:256 of the partner block; mine at 0:128 of my block.
        cfin = big.tile([DH + 1, H, PL], F32, tag="denSt")
        nc.vector.tensor_add(cfin[:], prtS[:, :, 0:PL],
                             prP[0:DH + 1, :, PL:PR_COL])
        den8c = big.tile([8, PL], F32, tag="den8l")
        nc.sync.dma_start(
            out=den8c[:],
            in_=cfin[DH:DH + 1, :, :].rearrange("p a b -> p (a b)"))
        rcp8c = big.tile([8, PL], F32, tag="den8r")
        nc.vector.reciprocal_approx_fast(rcp8c[:], den8c[:])
        OcT = pers.tile([128, KT, PL], BF16, tag="OcT")
        for h in range(H):
            hp, hr = h // 2, (h % 2) * DH
            rcpRow = pTmp.tile([1, SL], F32, tag="lnt0")
            nc.sync.dma_start(rcpRow[0:1, 0:PL], rcp8c[h:h + 1, :])
            psB = psT.tile([DH, PL], F32, tag="pt")
            nc.tensor.matmul(
                psB[:], lhsT=onesf.bitcast(F32R)[0:1, :],
                rhs=rcpRow.bitcast(F32R)[0:1, 0:PL], start=True, stop=True)
            nc.vector.tensor_mul(OcT[hr:hr + DH, hp, :], cfin[0:DH, h, :], psB[:])
        ps = psMM.tile([128, 512], F32, tag="mm")
        _mm_acc(nc, ps[:],
                [OcT[:, k, :] for k in range(KT)],
                [cWo_s[:, k, :] for k in range(KT)])
        outsb = pers.tile([128, D], F32, tag="outsb")
        nc.vector.tensor_add(outsb[:], ps[:], bcast["cbo"])
        nc.sync.dma_start(out_d[:], outsb[:])


def _layernorm(nc, pool, out_ap, in_ap, g_b, b_b, epsT):
    st = pool.tile([128, 6], F32, tag="ln_st")
    nc.vector.bn_stats(out=st[:], in_=in_ap)
    mv = pool.tile([128, 2], F32, tag="ln_mv")
    nc.vector.bn_aggr(out=mv[:], in_=st[:])
    sd = pool.tile([128, 1], F32, tag="ln_sd")
    nc.scalar.activation(sd[:], mv[:, 1:2], AF.Sqrt, bias=epsT[:])
    nc.vector.reciprocal(sd[:], sd[:])
    tmp = pool.tile([128, D], BF16, tag="ln_tmp")
    nc.vector.tensor_scalar(out=tmp[:], in0=in_ap, scalar1=mv[:, 0:1], scalar2=sd[:],
                            op0=ALU.subtract, op1=ALU.mult)
    nc.vector.tensor_mul(tmp[:], tmp[:], g_b[:])
    nc.vector.tensor_add(out_ap, tmp[:], b_b[:])


def _ngram_hashes(bytes_seq):
    """int64-wraparound n-gram hashes, mod V.  [B, S] -> [len(NGRAMS), B, S]"""
    b = bytes_seq.astype(np.int64)
    out = np.zeros((len(NGRAMS), b.shape[0], S), dtype=np.int64)
    for j, n in enumerate(NGRAMS):
        h = np.zeros_like(b)
        for k in range(n):
            shift = n - 1 - k
            mult = np.int64(256) ** k
            shifted = np.zeros_like(b)
            shifted[:, shift:] = b[:, : S - shift]
            h = h + shifted * mult
        h = np.where(np.arange(S)[None, :] >= (n - 1), h, 0)
        out[j] = h % V
    return out


_PROGRAM = None


def _get_program():
    global _PROGRAM
    if _PROGRAM is None:
        _PROGRAM = _build_program()
    return _PROGRAM


def _bf16(x):
    import ml_dtypes
    return np.asarray(x, dtype=np.float32).astype(ml_dtypes.bfloat16)


def make_in_maps(inputs):
    bytes_seq = np.asarray(inputs["bytes_seq"])
    patch_idx = np.asarray(inputs["patch_idx"])
    byte_emb = np.asarray(inputs["byte_emb"], dtype=np.float32)
    ngram_emb = np.asarray(inputs["ngram_emb"], dtype=np.float32)

    table = _bf16(np.concatenate(
        [byte_emb, ngram_emb.reshape(len(NGRAMS) * V, D)], axis=0))
    hashes = _ngram_hashes(bytes_seq)

    weights = {}
    for w in _W512 + ["W1", "W2"]:
        weights[w] = np.ascontiguousarray(_bf16(inputs[w]))
    for bv in ["sbq", "sbk", "cbq", "cbk", "b2", "b1"]:
        weights[bv] = np.ascontiguousarray(np.asarray(inputs[bv], dtype=np.float32))
    for bv in ["sbv", "sbo", "ln1g", "ln1b", "ln2g", "ln2b", "cbv", "cbo"]:
        weights[bv] = np.ascontiguousarray(_bf16(inputs[bv]))

    in_maps = []
    for c in range(N_CORES):
        b, hh = c // 2, c % 2
        tok0 = hh * SL
        p_ar = np.arange(128)[:, None]
        tt_ar = np.arange(TT_L)[None, :]
        tok = tok0 + tt_ar * 128 + p_ar          # [128, TT_L]
        idx = np.zeros((128, TT_L, NT), dtype=np.int32)
        idx[:, :, 0] = bytes_seq[b][tok].astype(np.int32)
        for j in range(len(NGRAMS)):
            idx[:, :, 1 + j] = (256 + j * V + hashes[j, b][tok]).astype(np.int32)

        # remote rank's rows in the kv blobs
        kr_idx = ((1 - hh) * 128 + np.arange(128)).astype(np.int32)[:, None]

        # patch-query routing for this batch (both halves' info is shared)
        g = patch_idx[b].astype(np.int64)        # [256] global patch tokens
        half_of = (g // SL).astype(np.int32)     # which rank owns each row
        slot = np.zeros(P, dtype=np.int32)       # slot in that rank's qcand
        cnt = [0, 0]
        for q in range(P):
            slot[q] = cnt[half_of[q]]
            cnt[half_of[q]] += 1
        # qc_idx: rows of local x2 this core ships (its residents, in slot order)
        qc = np.zeros(PR_COL, dtype=np.int32)
        for q in range(P):
            if half_of[q] == hh:
                qc[slot[q]] = g[q] % SL
        qc_idx = qc.reshape(2, 128).T.copy()     # [128, 2] (j-major columns)
        # qa_idx: assemble 256 rows own-half-queries-first from qx_all
        qa = np.zeros(PR_COL, dtype=np.int32)
        for i in range(PL):
            for j in range(2):
                q = (hh if j == 0 else 1 - hh) * PL + i
                qa[j * PL + i] = half_of[q] * PR_COL + slot[q]
        qa_idx = qa.reshape(2, 128).T.copy()
        # pr_idx: partner partial rows within each half blob (4 heads each)
        pr = np.zeros((128, H), dtype=np.int32)
        for r in range(128):
            for h2 in range(H):
                pr[r, h2] = ((1 - hh) * (PR_ROWS // 2) + (h2 % 4) * (DH + 1) + r
                             if r <= DH else 0)
        m = {"table": table, "idx": idx,
             "kr_idx": kr_idx.astype(np.int32),
             "qc_idx": np.ascontiguousarray(qc_idx),
             "qa_idx": np.ascontiguousarray(qa_idx),
             "pr_idx": pr}
        m.update(weights)
        in_maps.append(m)
    return in_maps


def assemble_output(results):
    out = np.zeros((B, P, D), dtype=np.float32)
    for c in range(N_CORES):
        b, hh = c // 2, c % 2
        out[b, hh * PL:(hh + 1) * PL, :] = results[c]["out"]
    return out


def kernel(**inputs):
    nc = _get_program()
    in_maps = make_in_maps(inputs)
    res = run_bass_kernel_spmd(nc, in_maps, core_ids=list(range(N_CORES)))
    return assemble_output(res.results)


if __name__ == "__main__":
    _build_program()
    print("program built OK")
